# revision 1
# baseline (speedup 1.0000x reference)
"""Trainium2 Bass kernel for nn_Mesh_Renderer: silhouette rasterizer.

Strategy: data-parallel over batch. Core b renders batch b's 64x64 silhouette
from 1280 triangles. Host-side work is layout only: slice per batch, gather
vertices[faces] (pure indexing, no arithmetic), transpose. All math (camera
transform, perspective divide, edge functions, coverage test, reduction) runs
on device.

Device pipeline per core:
  1. camera basis R from eye (look_at, mirrored op-for-op from the reference)
  2. v_cam = (verts - eye) @ R^T via PE matmuls  (verts pre-gathered per
     face-corner: 1280 faces x 4 corners (a,b,c,a) = 5120 columns)
  3. perspective: x_ndc = x / (z*tan + eps)
  4. edge coefficients per face-edge: e(x,y) = A*x + B*y + C
  5. rasterize 10 face-tiles of 128 faces x 4096 pixels: edge planes as
     K=9 bf16 PE matmuls (coefficients Dekker-split hi/mid/lo in bf16; the
     pixel basis [x,y,1] is exactly bf16, so all products are exact and the
     f32 PSUM accumulation gives f32-class e-values at 1 col/cycle), ACT
     Sign from PSUM -> bf16, min3/max3 chains in bf16 (2x DVE; note walrus
     rejects bf16 tensor_tensor on GPSIMD, so these stay on the DVE),
     acc += sign(min3) - sign(max3) in bf16 (exact small ints)
  6. final PE ones-matmul reduces faces; silhouette = count > -2*F, exact
     since invisible faces are rewritten to the never-covering plane set
     e = (-1, +1, +1) at the coefficient level
"""

import sys

if "/opt/trn_rl_repo" not in sys.path:
    sys.path.insert(0, "/opt/trn_rl_repo")

import numpy as np

import concourse.bacc as bacc
import concourse.tile as tile
from concourse import mybir
from concourse.bass_utils import run_bass_kernel_spmd

F32 = mybir.dt.float32
BF16 = mybir.dt.bfloat16
I32 = mybir.dt.int32
OP = mybir.AluOpType
AF = mybir.ActivationFunctionType

B, V, NF, IMG = 8, 642, 1280, 64
NPIX = IMG * IMG          # 4096
NTILE = NF // 128         # 10 face tiles
NCOL = NF * 4             # 5120 gathered corners (a, b, c, a)
EPS = 1e-8
# tan(deg2rad(15)) in float32, matching jnp.tan(jnp.deg2rad(float32(15)))
TAN_T = float(np.tan(np.deg2rad(np.float32(15.0)).astype(np.float32)))


def _normalize3(nc, pool, v, name):
    """v [1,3] f32 -> v / (||v|| + 1e-8), mirroring the reference formula."""
    sq = pool.tile([1, 3], F32, tag=f"{name}_sq")
    nc.vector.tensor_tensor(sq[:], v[:], v[:], OP.mult)
    s = pool.tile([1, 1], F32, tag=f"{name}_s")
    nc.vector.tensor_reduce(s[:], sq[:], mybir.AxisListType.X, OP.add)
    n = pool.tile([1, 1], F32, tag=f"{name}_n")
    nc.scalar.activation(n[:], s[:], AF.Sqrt)
    # Newton refine sqrt: n1 = 0.5*(n + s/n)
    rn = pool.tile([1, 1], F32, tag=f"{name}_rn")
    nc.vector.reciprocal(rn[:], n[:])
    t = pool.tile([1, 1], F32, tag=f"{name}_t")
    nc.vector.tensor_tensor(t[:], s[:], rn[:], OP.mult)
    t2 = pool.tile([1, 1], F32, tag=f"{name}_t2")
    nc.vector.tensor_tensor(t2[:], n[:], t[:], OP.add)
    n1 = pool.tile([1, 1], F32, tag=f"{name}_n1")
    nc.vector.tensor_scalar(n1[:], t2[:], 0.5, None, OP.mult)
    d = pool.tile([1, 1], F32, tag=f"{name}_d")
    nc.vector.tensor_scalar(d[:], n1[:], EPS, None, OP.add)
    r = pool.tile([1, 1], F32, tag=f"{name}_r")
    nc.vector.reciprocal(r[:], d[:])
    # Newton refine recip: r1 = r*(2 - d*r)
    u = pool.tile([1, 1], F32, tag=f"{name}_u")
    nc.vector.tensor_tensor(u[:], d[:], r[:], OP.mult)
    u2 = pool.tile([1, 1], F32, tag=f"{name}_u2")
    nc.vector.tensor_scalar(u2[:], u[:], -1.0, 2.0, OP.mult, OP.add)
    r1 = pool.tile([1, 1], F32, tag=f"{name}_r1")
    nc.vector.tensor_tensor(r1[:], r[:], u2[:], OP.mult)
    out = pool.tile([1, 3], F32, tag=f"{name}_out")
    nc.vector.tensor_scalar(out[:], v[:], r1[:], None, OP.mult)
    return out


def _cross3(nc, pool, a, b, name):
    """cross(a, b) for [1,3] tiles via duplicated [1,6] buffers."""
    a2 = pool.tile([1, 6], F32, tag=f"{name}_a2")
    nc.vector.tensor_copy(a2[:, 0:3], a[:])
    nc.vector.tensor_copy(a2[:, 3:6], a[:])
    b2 = pool.tile([1, 6], F32, tag=f"{name}_b2")
    nc.vector.tensor_copy(b2[:, 0:3], b[:])
    nc.vector.tensor_copy(b2[:, 3:6], b[:])
    m1 = pool.tile([1, 3], F32, tag=f"{name}_m1")
    nc.vector.tensor_tensor(m1[:], a2[:, 1:4], b2[:, 2:5], OP.mult)
    m2 = pool.tile([1, 3], F32, tag=f"{name}_m2")
    nc.vector.tensor_tensor(m2[:], a2[:, 2:5], b2[:, 1:4], OP.mult)
    out = pool.tile([1, 3], F32, tag=f"{name}_out")
    nc.vector.tensor_tensor(out[:], m1[:], m2[:], OP.subtract)
    return out


def build_kernel(ctx, tc):
    nc = tc.nc
    vgt_d = nc.dram_tensor("vgt", [3, NCOL], F32, kind="ExternalInput")
    eye_d = nc.dram_tensor("eye", [3], F32, kind="ExternalInput")
    sil_d = nc.dram_tensor("sil", [NPIX], F32, kind="ExternalOutput")

    cpool = ctx.enter_context(tc.tile_pool(name="cam", bufs=1))
    ppool = ctx.enter_context(tc.tile_pool(name="proj", bufs=1))
    gpool = ctx.enter_context(tc.tile_pool(name="grid", bufs=1))

    # ---- camera basis (partition 0, tiny tiles) ----
    eyeR = cpool.tile([1, 3], F32)
    nc.sync.dma_start(eyeR[:], eye_d.ap())
    eT = cpool.tile([3, 1], F32)
    nc.sync.dma_start(eT[:], eye_d.ap())

    nege = cpool.tile([1, 3], F32)
    nc.vector.tensor_scalar(nege[:], eyeR[:], -1.0, None, OP.mult)
    z_ax = _normalize3(nc, cpool, nege, "nz")

    xr = cpool.tile([1, 3], F32)
    nc.vector.memset(xr[:], 0.0)
    nc.vector.tensor_copy(xr[:, 0:1], z_ax[:, 2:3])
    nc.vector.tensor_scalar(xr[:, 2:3], z_ax[:, 0:1], -1.0, None, OP.mult)
    x_ax = _normalize3(nc, cpool, xr, "nx")

    yr = _cross3(nc, cpool, z_ax, x_ax, "cy")
    y_ax = _normalize3(nc, cpool, yr, "ny")

    # RT[c, d] = R[d, c]; column d of RT = axis row d
    rt = cpool.tile([3, 3], F32)
    for d, axis in enumerate([x_ax, y_ax, z_ax]):
        nc.sync.dma_start(rt[:, d : d + 1], axis[:])

    # ---- projection of 5120 gathered corners ----
    vca = ppool.tile([128, 120], F32)  # [p, (chunk c, coord d)]
    with tc.tile_pool(name="vg", bufs=1) as vgp, \
         tc.tile_pool(name="pvc", bufs=1, space="PSUM") as psvc:
        vgt = vgp.tile([3, NCOL], F32)
        nc.sync.dma_start(vgt[:], vgt_d.ap())
        vme = vgp.tile([3, NCOL], F32)
        nc.vector.tensor_scalar(vme[:], vgt[:], eT[:], None, OP.subtract)
        vcp = psvc.tile([128, 120], F32)
        for c in range(40):
            nc.tensor.matmul(
                vcp[:, 3 * c : 3 * c + 3],
                vme[:, 128 * c : 128 * (c + 1)],
                rt[:],
                start=True,
                stop=True,
            )
        nc.vector.tensor_copy(vca[:], vcp[:])

    vcav = vca[:].rearrange("p (c d) -> p c d", d=3)
    vx, vy, vz = vcav[:, :, 0], vcav[:, :, 1], vcav[:, :, 2]

    dn = ppool.tile([128, 40], F32)
    nc.vector.tensor_scalar(dn[:], vz, TAN_T, EPS, OP.mult, OP.add)
    rc0 = ppool.tile([128, 40], F32)
    nc.vector.reciprocal(rc0[:], dn[:])
    t = ppool.tile([128, 40], F32)
    nc.vector.tensor_tensor(t[:], dn[:], rc0[:], OP.mult)
    t2 = ppool.tile([128, 40], F32)
    nc.vector.tensor_scalar(t2[:], t[:], -1.0, 2.0, OP.mult, OP.add)
    rc = ppool.tile([128, 40], F32)
    nc.vector.tensor_tensor(rc[:], rc0[:], t2[:], OP.mult)

    xn = ppool.tile([128, 40], F32)
    nc.vector.tensor_tensor(xn[:], vx, rc[:], OP.mult)
    yn = ppool.tile([128, 40], F32)
    nc.vector.tensor_tensor(yn[:], vy, rc[:], OP.mult)

    # ---- edge coefficients: e = A*x + B*y + C per (face, edge) ----
    # Visibility is folded into the coefficients: invisible faces get their
    # projected coords zeroed (A=B=C=0) and then per-edge constant planes
    # e0=-1, e1=e2=+1, which cover nothing under the pos/neg test.
    vz4 = vca[:].rearrange("p (ft k d) -> p ft k d", k=4, d=3)
    mz1 = ppool.tile([128, 10], F32)
    nc.vector.tensor_tensor(mz1[:], vz4[:, :, 0, 2], vz4[:, :, 1, 2], OP.min)
    mz = ppool.tile([128, 10], F32)
    nc.vector.tensor_tensor(mz[:], mz1[:], vz4[:, :, 2, 2], OP.min)
    vg = ppool.tile([128, 10], F32)
    nc.vector.tensor_scalar(vg[:], mz[:], 0.0, None, OP.is_gt)

    xn2 = ppool.tile([128, 40], F32)
    nc.vector.tensor_tensor(
        xn2[:].rearrange("p (ft k) -> p ft k", k=4), 
        xn[:].rearrange("p (ft k) -> p ft k", k=4),
        vg[:].unsqueeze(2).broadcast_to([128, 10, 4]), OP.mult)
    yn2 = ppool.tile([128, 40], F32)
    nc.vector.tensor_tensor(
        yn2[:].rearrange("p (ft k) -> p ft k", k=4),
        yn[:].rearrange("p (ft k) -> p ft k", k=4),
        vg[:].unsqueeze(2).broadcast_to([128, 10, 4]), OP.mult)

    xnv = xn2[:].rearrange("p (ft k) -> p ft k", k=4)
    ynv = yn2[:].rearrange("p (ft k) -> p ft k", k=4)
    # CAB[p, (ft, k, c)]: c = 0/1/2 -> A/B/C for edge k of face 128*ft+p
    CAB = ppool.tile([128, 90], F32)
    CABv = CAB[:].rearrange("p (ft k c) -> p ft k c", k=3, c=3)
    nc.vector.tensor_tensor(CABv[:, :, :, 0], ynv[:, :, 0:3], ynv[:, :, 1:4],
                            OP.subtract)
    nc.vector.tensor_tensor(CABv[:, :, :, 1], xnv[:, :, 1:4], xnv[:, :, 0:3],
                            OP.subtract)
    p1 = ppool.tile([128, 30], F32)
    nc.vector.tensor_tensor(p1[:].rearrange("p (ft k) -> p ft k", k=3),
                            xnv[:, :, 0:3], ynv[:, :, 1:4], OP.mult)
    p2 = ppool.tile([128, 30], F32)
    nc.vector.tensor_tensor(p2[:].rearrange("p (ft k) -> p ft k", k=3),
                            ynv[:, :, 0:3], xnv[:, :, 1:4], OP.mult)
    c0 = ppool.tile([128, 30], F32)
    nc.vector.tensor_tensor(c0[:], p1[:], p2[:], OP.subtract)
    # C offset for invisible faces: (1-vg) * (-1, +1, +1)
    pat = ppool.tile([128, 3], F32)
    nc.vector.memset(pat[:, 0:1], -1.0)
    nc.vector.memset(pat[:, 1:3], 1.0)
    ivg = ppool.tile([128, 10], F32)
    nc.vector.tensor_scalar(ivg[:], vg[:], -1.0, 1.0, OP.mult, OP.add)
    off = ppool.tile([128, 30], F32)
    nc.vector.tensor_tensor(off[:].rearrange("p (ft k) -> p ft k", k=3),
                            ivg[:].unsqueeze(2).broadcast_to([128, 10, 3]),
                            pat[:].unsqueeze(1).broadcast_to([128, 10, 3]),
                            OP.mult)
    nc.vector.tensor_tensor(CABv[:, :, :, 2],
                            c0[:].rearrange("p (ft k) -> p ft k", k=3),
                            off[:].rearrange("p (ft k) -> p ft k", k=3), OP.add)

    # 3-way Dekker split of coefficients into bf16 (hi+mid+lo ~ f32-exact;
    # pixel-grid values are exactly bf16, so bf16 x bf16 products are exact
    # and the K=9 matmul accumulates them in f32 PSUM)
    CAB27 = ppool.tile([128, 270], F32)  # col = ft*27 + k*9 + s*3 + c
    c27 = CAB27[:].rearrange("p (ft k s c) -> p ft k s c", k=3, s=3, c=3)
    hib = ppool.tile([128, 90], BF16)
    nc.vector.tensor_copy(hib[:], CAB[:])                 # hi (bf16 rounded)
    nc.vector.tensor_copy(c27[:, :, :, 0],
                          hib[:].rearrange("p (ft k c) -> p ft k c", k=3, c=3))
    r1 = ppool.tile([128, 90], F32)
    nc.vector.tensor_tensor(r1[:], CAB[:], c27[:, :, :, 0].copy(), OP.subtract)
    mib = ppool.tile([128, 90], BF16)
    nc.vector.tensor_copy(mib[:], r1[:])                  # mid
    nc.vector.tensor_copy(c27[:, :, :, 1],
                          mib[:].rearrange("p (ft k c) -> p ft k c", k=3, c=3))
    r2 = ppool.tile([128, 90], F32)
    nc.vector.tensor_tensor(r2[:], r1[:], c27[:, :, :, 1].copy(), OP.subtract)
    lob = ppool.tile([128, 90], BF16)
    nc.vector.tensor_copy(lob[:], r2[:])                  # lo
    nc.vector.tensor_copy(c27[:, :, :, 2],
                          lob[:].rearrange("p (ft k c) -> p ft k c", k=3, c=3))

    # ---- pixel grids and basis G = [x; y; 1] over raster order (i, j) ----
    it32 = gpool.tile([128, IMG], I32)
    nc.gpsimd.iota(it32[:], pattern=[[1, IMG]], base=0, channel_multiplier=0)
    itf = gpool.tile([128, IMG], F32)
    nc.vector.tensor_copy(itf[:], it32[:])
    xg = gpool.tile([128, IMG], F32)  # x_j = j/32 - 63/64 (exact)
    nc.vector.tensor_scalar(xg[:], itf[:], 1.0 / 32.0, -63.0 / 64.0, OP.mult, OP.add)
    yg = gpool.tile([128, IMG], F32)  # y_i = -x_i
    nc.vector.tensor_scalar(yg[:], xg[:], -1.0, None, OP.mult)
    ones_bf = gpool.tile([128, 1], BF16)
    nc.vector.memset(ones_bf[:], 1.0)
    # stage basis rows on partition 0 (compute engines cannot start at
    # partition>0), then one DMA redistributes to [9, NPIX] bf16 (3 copies of
    # x,y,1 to pair with the hi/mid/lo coefficient rows)
    G9 = gpool.tile([9, NPIX], BF16)
    gst = gpool.tile([1, 3 * NPIX], BF16)
    gsv = gst[:].rearrange("p (r i j) -> p r i j", r=3, i=IMG)
    nc.vector.tensor_copy(gsv[:, 0], xg[0:1, :].unsqueeze(1)
                          .broadcast_to([1, IMG, IMG]))
    nc.vector.tensor_copy(gsv[:, 1], yg[0:1, :].unsqueeze(2)
                          .broadcast_to([1, IMG, IMG]))
    nc.vector.memset(gst[:, 2 * NPIX :], 1.0)
    for rep in range(3):
        nc.sync.dma_start(G9[3 * rep : 3 * rep + 3, :], gst[:])

    # identity for PE transposes
    iop = gpool.tile([128, 1], I32)
    nc.gpsimd.iota(iop[:], pattern=[[1, 1]], base=0, channel_multiplier=1)
    iopf = gpool.tile([128, 1], F32)
    nc.vector.tensor_copy(iopf[:], iop[:])
    iof = gpool.tile([128, 128], I32)
    nc.gpsimd.iota(iof[:], pattern=[[1, 128]], base=0, channel_multiplier=0)
    ioff = gpool.tile([128, 128], F32)
    nc.vector.tensor_copy(ioff[:], iof[:])
    idm = gpool.tile([128, 128], F32)
    nc.vector.tensor_scalar(idm[:], ioff[:], iopf[:], None, OP.is_equal)

    # ---- coefficient transposes: TC[:, (ft*3+k)*128 : +128] = [3, 128] lhsT ----
    TCf = gpool.tile([9, NF * 3], F32)
    with tc.tile_pool(name="ptp", bufs=2, space="PSUM") as ptp:
        for ft in range(NTILE):
            for k in range(3):
                tps = ptp.tile([9, 128], F32, tag="tps")
                nc.tensor.matmul(
                    tps[:], CAB27[:, 27 * ft + 9 * k : 27 * ft + 9 * k + 9],
                    idm[:], start=True, stop=True)
                nc.vector.tensor_copy(
                    TCf[:, (ft * 3 + k) * 128 : (ft * 3 + k + 1) * 128], tps[:])
    TC = gpool.tile([9, NF * 3], BF16)
    nc.vector.tensor_copy(TC[:], TCf[:])

    # ---- rasterization ----
    # Per (face-tile, half, edge): PE matmul (coef lhsT [3,128] x G-half
    # [3,2048]) -> e-plane in PSUM, ACT Sign -> bf16 SBUF. Then smin chain on
    # DVE, smax chain on GPSIMD, acc += smin - smax (bf16, exact ints).
    HALF = NPIX // 2
    spool = ctx.enter_context(tc.tile_pool(name="s3", bufs=2))
    mpool = ctx.enter_context(tc.tile_pool(name="mm", bufs=8))
    apool = ctx.enter_context(tc.tile_pool(name="accp", bufs=4))
    accs = [None, None]
    with tc.tile_pool(name="pe3", bufs=2, space="PSUM") as psE:
        for ft in range(NTILE):
            for h in range(2):
                s3 = spool.tile([128, 3 * HALF], BF16, tag="s3")
                for k in range(3):
                    eps = psE.tile([128, HALF], F32, tag="eps")
                    lhsT = TC[:, (ft * 3 + k) * 128 : (ft * 3 + k + 1) * 128]
                    for q in range(HALF // 512):
                        nc.tensor.matmul(
                            eps[:, 512 * q : 512 * (q + 1)], lhsT,
                            G9[:, HALF * h + 512 * q : HALF * h + 512 * (q + 1)],
                            start=True, stop=True)
                    nc.scalar.activation(s3[:, HALF * k : HALF * (k + 1)],
                                         eps[:], AF.Sign)
                s3r = s3[:].rearrange("p (k x) -> p k x", k=3)
                sm1 = mpool.tile([128, HALF], BF16, tag="mm")
                nc.vector.tensor_tensor(sm1[:], s3r[:, 0], s3r[:, 1], OP.min)
                smin = mpool.tile([128, HALF], BF16, tag="mm")
                nc.vector.tensor_tensor(smin[:], sm1[:], s3r[:, 2], OP.min)
                sM1 = mpool.tile([128, HALF], BF16, tag="mm")
                nc.vector.tensor_tensor(sM1[:], s3r[:, 0], s3r[:, 1], OP.max)
                smax = mpool.tile([128, HALF], BF16, tag="mm")
                nc.vector.tensor_tensor(smax[:], sM1[:], s3r[:, 2], OP.max)
                if accs[h] is None:
                    a = apool.tile([128, HALF], BF16, tag=f"a{h}")
                    nc.vector.tensor_tensor(a[:], smin[:], smax[:], OP.subtract)
                    accs[h] = a
                else:
                    d = mpool.tile([128, HALF], BF16, tag="mm")
                    nc.vector.tensor_tensor(d[:], smin[:], smax[:], OP.subtract)
                    a = apool.tile([128, HALF], BF16, tag=f"a{h}")
                    nc.vector.tensor_tensor(a[:], accs[h][:], d[:], OP.add)
                    accs[h] = a

    # ---- reduce over faces, threshold T > -2F, output ----
    pscnt = ctx.enter_context(tc.tile_pool(name="pcnt", bufs=1, space="PSUM"))
    cnt = pscnt.tile([1, NPIX], F32, tag="cnt")
    for h in range(2):
        for q in range(HALF // 512):
            off2 = HALF * h + 512 * q
            nc.tensor.matmul(cnt[:, off2 : off2 + 512], ones_bf[:],
                             accs[h][:, 512 * q : 512 * (q + 1)],
                             start=True, stop=True)
    silb = gpool.tile([1, NPIX], F32)
    nc.vector.tensor_scalar(silb[:], cnt[:], -2.0 * NF, None, OP.is_gt)
    nc.sync.dma_start(sil_d.ap(), silb[:])


_NC = None


def _get_program():
    global _NC
    if _NC is None:
        nc = bacc.Bacc(
            "TRN2",
            target_bir_lowering=False,
            debug=False,
            enable_asserts=False,
            num_devices=B,
        )
        from contextlib import ExitStack

        with tile.TileContext(nc) as tc:
            with ExitStack() as ctx:
                build_kernel(ctx, tc)
        nc.compile()
        _NC = nc
    return _NC


def _host_layout(vertices, faces):
    """Pure indexing: gather per-face-corner vertices, layout [3, 5120] where
    column n = ft*512 + k*128 + p holds corner k of face ft*128+p."""
    faces4 = np.concatenate([faces, faces[:, :1]], axis=1)  # [1280, 4]
    vidx = faces4.reshape(NTILE, 128, 4).transpose(0, 2, 1).reshape(-1)  # [5120]
    out = []
    for b in range(B):
        vg = vertices[b][vidx]  # [5120, 3]
        out.append(np.ascontiguousarray(vg.T.astype(np.float32)))
    return out


def kernel(vertices, viewpoints, faces, img_size):
    vertices = np.asarray(vertices, dtype=np.float32)
    viewpoints = np.asarray(viewpoints, dtype=np.float32)
    faces = np.asarray(faces, dtype=np.int32)
    assert int(img_size) == IMG and vertices.shape == (B, V, 3)

    nc = _get_program()
    vgts = _host_layout(vertices, faces)
    in_maps = [
        {"vgt": vgts[b], "eye": np.ascontiguousarray(viewpoints[b])}
        for b in range(B)
    ]
    res = run_bass_kernel_spmd(nc, in_maps, core_ids=list(range(B)))
    sil = np.stack([res.results[b]["sil"] for b in range(B)])  # [8, 4096]
    return sil.reshape(B, 1, IMG, IMG).astype(np.float32)


if __name__ == "__main__":
    # quick self-exercise with random data
    rng = np.random.default_rng(0)
    verts = rng.standard_normal((B, V, 3), dtype=np.float32) * 0.5
    vps = rng.standard_normal((B, 3), dtype=np.float32)
    fcs = rng.integers(0, V, (NF, 3), dtype=np.int32)
    out = kernel(verts, vps, fcs, IMG)
    print(out.shape, out.sum())



# revision 5
# speedup vs baseline: 2.3343x; 2.3343x over previous
"""Trainium2 Bass kernel for nn_Mesh_Renderer: silhouette via scanline intervals.

Data-parallel over batch (core b renders view b). Host work is layout only:
gather vertices[faces] into [3, 5120] (+ ones row), transpose the returned
image. All math on device.

Device algorithm (per core):
  1. look_at camera basis from eye; projection folded as [w;1]^T @ [R^T; -R@eye]
     (40 K=4 f32 matmuls), perspective divide -> per-corner (xn, yn) [128, 40].
  2. Edge coefficients per (face, edge): e = A x + B y + C. For each pixel row
     y_i the face coverage in x is an interval [lo, hi]:
       t_k(i) = -(B_k y_i + C_k)/A_k ; edge k bounds from below iff
       sign(2*area)*A_k > 0. lo = max over lower-edges, hi = min over upper.
     Invisible/degenerate faces forced to the contributes-nothing interval via
     +-BIG offsets folded into the per-edge (u, v) = (-B/A, -C/A) small tiles.
     Empty rows canonicalized with hi' = max(hi, lo) (point interval).
     All interval math on [128, 30]/[128, 1920] tiles, endpoints in bf16.
  3. Raster: count(i,j) = sum_f([x_j >= lo] + [x_j <= hi]) = F + #covering.
     One DVE is_ge over [128, 8192] per 128-face tile computes both compares
     ([x | -x] vs [lo | -hi] with broadcast over j); PE ones-matmuls accumulate
     over faces into PSUM cnt8 [8, 512] (sliding-onehot lhsT selects the row).
  4. silhouette = cnt > 2F ... cnt >= F+1; DMA out; host transposes (j,i)->(i,j).
"""

import sys

if "/opt/trn_rl_repo" not in sys.path:
    sys.path.insert(0, "/opt/trn_rl_repo")

import numpy as np

import concourse.bacc as bacc
import concourse.tile as tile
from concourse import mybir
from concourse.bass_utils import run_bass_kernel_spmd

F32 = mybir.dt.float32
BF16 = mybir.dt.bfloat16
I32 = mybir.dt.int32
OP = mybir.AluOpType
AF = mybir.ActivationFunctionType

B, V, NF, IMG = 8, 642, 1280, 64
NPIX = IMG * IMG          # 4096
NTILE = NF // 128         # 10 face tiles
NCOL = NF * 4             # 5120 gathered corners (a, b, c, a)
EPS = 1e-8
BIG = 1.0e30
TAN_T = float(np.tan(np.deg2rad(np.float32(15.0)).astype(np.float32)))


def _normalize3(nc, pool, v, name):
    """v [1,3] f32 -> v / (||v|| + 1e-8)."""
    sq = pool.tile([1, 3], F32, tag=f"{name}_sq")
    nc.vector.tensor_tensor(sq[:], v[:], v[:], OP.mult)
    s = pool.tile([1, 1], F32, tag=f"{name}_s")
    nc.vector.tensor_reduce(s[:], sq[:], mybir.AxisListType.X, OP.add)
    n = pool.tile([1, 1], F32, tag=f"{name}_n")
    nc.scalar.activation(n[:], s[:], AF.Sqrt)
    d = pool.tile([1, 1], F32, tag=f"{name}_d")
    nc.vector.tensor_scalar(d[:], n[:], EPS, None, OP.add)
    r = pool.tile([1, 1], F32, tag=f"{name}_r")
    nc.vector.reciprocal(r[:], d[:])
    out = pool.tile([1, 3], F32, tag=f"{name}_out")
    nc.vector.tensor_scalar(out[:], v[:], r[:], None, OP.mult)
    return out


def _cross3(nc, pool, a, b, name):
    """cross(a, b) for [1,3] tiles via duplicated [1,6] buffers."""
    a2 = pool.tile([1, 6], F32, tag=f"{name}_a2")
    nc.vector.tensor_copy(a2[:, 0:3], a[:])
    nc.vector.tensor_copy(a2[:, 3:6], a[:])
    b2 = pool.tile([1, 6], F32, tag=f"{name}_b2")
    nc.vector.tensor_copy(b2[:, 0:3], b[:])
    nc.vector.tensor_copy(b2[:, 3:6], b[:])
    m1 = pool.tile([1, 3], F32, tag=f"{name}_m1")
    nc.vector.tensor_tensor(m1[:], a2[:, 1:4], b2[:, 2:5], OP.mult)
    m2 = pool.tile([1, 3], F32, tag=f"{name}_m2")
    nc.vector.tensor_tensor(m2[:], a2[:, 2:5], b2[:, 1:4], OP.mult)
    out = pool.tile([1, 3], F32, tag=f"{name}_out")
    nc.vector.tensor_tensor(out[:], m1[:], m2[:], OP.subtract)
    return out


def build_kernel(ctx, tc):
    nc = tc.nc
    vgt_d = nc.dram_tensor("vgt4", [4, NCOL], F32, kind="ExternalInput")
    eye_d = nc.dram_tensor("eye", [3], F32, kind="ExternalInput")
    sil_d = nc.dram_tensor("sil", [NPIX], F32, kind="ExternalOutput")

    cpool = ctx.enter_context(tc.tile_pool(name="cam", bufs=1))
    ppool = ctx.enter_context(tc.tile_pool(name="proj", bufs=1))
    gpool = ctx.enter_context(tc.tile_pool(name="grid", bufs=1))

    # ---- input DMAs ----
    eyeR = cpool.tile([1, 3], F32)
    nc.sync.dma_start(eyeR[:], eye_d.ap())
    eT = cpool.tile([3, 1], F32)
    nc.sync.dma_start(eT[:], eye_d.ap())
    vgt = gpool.tile([4, NCOL], F32)
    nc.sync.dma_start(vgt[:], vgt_d.ap())

    # ---- camera basis (partition 0, tiny tiles) ----
    nege = cpool.tile([1, 3], F32)
    nc.vector.tensor_scalar(nege[:], eyeR[:], -1.0, None, OP.mult)
    z_ax = _normalize3(nc, cpool, nege, "nz")

    xr = cpool.tile([1, 3], F32)
    nc.vector.memset(xr[:], 0.0)
    nc.vector.tensor_copy(xr[:, 0:1], z_ax[:, 2:3])
    nc.vector.tensor_scalar(xr[:, 2:3], z_ax[:, 0:1], -1.0, None, OP.mult)
    x_ax = _normalize3(nc, cpool, xr, "nx")

    yr = _cross3(nc, cpool, z_ax, x_ax, "cy")
    y_ax = _normalize3(nc, cpool, yr, "ny")

    # rt4: rows 0-2 = R^T (column d = axis row d), row 3 = -(eye^T @ R^T)
    rt4 = cpool.tile([4, 3], F32)
    for d, axis in enumerate([x_ax, y_ax, z_ax]):
        nc.sync.dma_start(rt4[0:3, d : d + 1], axis[:])
    with tc.tile_pool(name="prey", bufs=1, space="PSUM") as psr:
        reyep = psr.tile([1, 3], F32)
        nc.tensor.matmul(reyep[:], eT[:], rt4[0:3, :], start=True, stop=True)
        nreye = cpool.tile([1, 3], F32)
        nc.vector.tensor_scalar(nreye[:], reyep[:], -1.0, None, OP.mult)
    nc.sync.dma_start(rt4[3:4, :], nreye[:])

    # ---- projection: vca[p, (c, d)] = [w;1]^T @ rt4, c = ft*4 + k ----
    vca = ppool.tile([128, 120], F32)
    with tc.tile_pool(name="pvc", bufs=1, space="PSUM") as psvc:
        vcp = psvc.tile([128, 120], F32)
        for c in range(40):
            nc.tensor.matmul(
                vcp[:, 3 * c : 3 * c + 3],
                vgt[:, 128 * c : 128 * (c + 1)],
                rt4[:],
                start=True,
                stop=True,
            )
        nc.vector.tensor_copy(vca[:], vcp[:])

    vcav = vca[:].rearrange("p (c d) -> p c d", d=3)
    vx, vy, vz = vcav[:, :, 0], vcav[:, :, 1], vcav[:, :, 2]

    # perspective divide (raw reciprocal; interval margins tolerate ~3e-3)
    dn = ppool.tile([128, 40], F32)
    nc.vector.tensor_scalar(dn[:], vz, TAN_T, EPS, OP.mult, OP.add)
    rc = ppool.tile([128, 40], F32)
    nc.vector.reciprocal(rc[:], dn[:])
    xn = ppool.tile([128, 40], F32)
    nc.vector.tensor_tensor(xn[:], vx, rc[:], OP.mult)
    yn = ppool.tile([128, 40], F32)
    nc.vector.tensor_tensor(yn[:], vy, rc[:], OP.mult)

    # visibility: all corner z > 0
    vz4 = vca[:].rearrange("p (ft k d) -> p ft k d", k=4, d=3)
    mz1 = ppool.tile([128, 10], F32)
    nc.vector.tensor_tensor(mz1[:], vz4[:, :, 0, 2], vz4[:, :, 1, 2], OP.min)
    mz = ppool.tile([128, 10], F32)
    nc.vector.tensor_tensor(mz[:], mz1[:], vz4[:, :, 2, 2], OP.min)
    vg = ppool.tile([128, 10], F32)
    nc.vector.tensor_scalar(vg[:], mz[:], 0.0, None, OP.is_gt)

    # ---- edge coefficients [128, 30] in (ft, k) layout ----
    xn4 = xn[:].rearrange("p (ft k) -> p ft k", k=4)
    yn4 = yn[:].rearrange("p (ft k) -> p ft k", k=4)
    xk, xk1 = xn4[:, :, 0:3], xn4[:, :, 1:4]
    yk, yk1 = yn4[:, :, 0:3], yn4[:, :, 1:4]

    def t30(name):
        return ppool.tile([128, 30], F32, name=name, tag=name)

    A = t30("A")
    Av = A[:].rearrange("p (ft k) -> p ft k", k=3)
    nc.vector.tensor_tensor(Av, yk, yk1, OP.subtract)
    Bc = t30("Bc")
    Bv = Bc[:].rearrange("p (ft k) -> p ft k", k=3)
    nc.vector.tensor_tensor(Bv, xk1, xk, OP.subtract)
    p1 = t30("p1")
    nc.vector.tensor_tensor(p1[:].rearrange("p (ft k) -> p ft k", k=3), xk, yk1,
                            OP.mult)
    p2 = t30("p2")
    nc.vector.tensor_tensor(p2[:].rearrange("p (ft k) -> p ft k", k=3), yk, xk1,
                            OP.mult)
    C = t30("C")
    nc.vector.tensor_tensor(C[:], p1[:], p2[:], OP.subtract)

    Cv = C[:].rearrange("p (ft k) -> p ft k", k=3)
    S1 = ppool.tile([128, 10], F32, tag="S1")
    nc.vector.tensor_tensor(S1[:], Cv[:, :, 0], Cv[:, :, 1], OP.add)
    S = ppool.tile([128, 10], F32, tag="S")
    nc.vector.tensor_tensor(S[:], S1[:], Cv[:, :, 2], OP.add)

    w = t30("w")
    nc.vector.tensor_tensor(w[:].rearrange("p (ft k) -> p ft k", k=3), Av,
                            S[:].unsqueeze(2).broadcast_to([128, 10, 3]),
                            OP.mult)
    mpos = t30("mpos")
    nc.vector.tensor_scalar(mpos[:], w[:], 0.0, None, OP.is_gt)
    mneg = t30("mneg")
    nc.vector.tensor_scalar(mneg[:], w[:], 0.0, None, OP.is_lt)

    sne = ppool.tile([128, 10], F32, tag="sne")
    nc.vector.tensor_scalar(sne[:], S[:], 0.0, None, OP.not_equal)
    visq = ppool.tile([128, 10], F32, tag="visq")
    nc.vector.tensor_tensor(visq[:], vg[:], sne[:], OP.mult)
    # ivq = (1 - visq) * 2BIG ; ivqN = -(ivq)
    ivq = ppool.tile([128, 10], F32, tag="ivq")
    nc.vector.tensor_scalar(ivq[:], visq[:], -2.0 * BIG, 2.0 * BIG, OP.mult,
                            OP.add)
    ivqN = ppool.tile([128, 10], F32, tag="ivqN")
    nc.vector.tensor_scalar(ivqN[:], visq[:], 2.0 * BIG, -2.0 * BIG, OP.mult,
                            OP.add)

    iseq = t30("iseq")
    nc.vector.tensor_scalar(iseq[:], A[:], 0.0, None, OP.is_equal)
    Asafe = t30("Asafe")
    nc.vector.tensor_tensor(Asafe[:], A[:], iseq[:], OP.add)
    r0 = t30("r0")
    nc.vector.reciprocal(r0[:], Asafe[:])
    nr = t30("nr")
    nc.vector.tensor_scalar(nr[:], r0[:], -1.0, None, OP.mult)
    u = t30("u")
    nc.vector.tensor_tensor(u[:], Bc[:], nr[:], OP.mult)
    v = t30("v")
    nc.vector.tensor_tensor(v[:], C[:], nr[:], OP.mult)

    # lower-bound side: ulo = u*mpos ; vlo = v*mpos - BIG*(1-mpos) + ivq
    ulo = t30("ulo")
    nc.vector.tensor_tensor(ulo[:], u[:], mpos[:], OP.mult)
    offlo = t30("offlo")
    nc.vector.tensor_scalar(offlo[:], mpos[:], BIG, -BIG, OP.mult, OP.add)
    vlo1 = t30("vlo1")
    nc.vector.tensor_tensor(vlo1[:], v[:], mpos[:], OP.mult)
    vlo2 = t30("vlo2")
    nc.vector.tensor_tensor(vlo2[:], vlo1[:], offlo[:], OP.add)
    vlo = t30("vlo")
    nc.vector.tensor_tensor(vlo[:].rearrange("p (ft k) -> p ft k", k=3),
                            vlo2[:].rearrange("p (ft k) -> p ft k", k=3),
                            ivq[:].unsqueeze(2).broadcast_to([128, 10, 3]),
                            OP.add)

    # negated upper side: tnh = -thi: unh = -u*mneg ; vnh = -v*mneg -
    # BIG*(1-mneg) - ivq  (masked edges -> -BIG; invisible -> ~-2BIG)
    mnegN = t30("mnegN")
    nc.vector.tensor_scalar(mnegN[:], mneg[:], -1.0, None, OP.mult)
    unh = t30("unh")
    nc.vector.tensor_tensor(unh[:], u[:], mnegN[:], OP.mult)
    vnh1 = t30("vnh1")
    nc.vector.tensor_tensor(vnh1[:], v[:], mnegN[:], OP.mult)
    offnh = t30("offnh")
    nc.vector.tensor_scalar(offnh[:], mneg[:], BIG, -BIG, OP.mult, OP.add)
    vnh2 = t30("vnh2")
    nc.vector.tensor_tensor(vnh2[:], vnh1[:], offnh[:], OP.add)
    vnh = t30("vnh")
    nc.vector.tensor_tensor(vnh[:].rearrange("p (ft k) -> p ft k", k=3),
                            vnh2[:].rearrange("p (ft k) -> p ft k", k=3),
                            ivqN[:].unsqueeze(2).broadcast_to([128, 10, 3]),
                            OP.add)

    # bf16 conversions of the four coefficient tiles
    ulob = ppool.tile([128, 30], BF16, tag="ulob")
    nc.vector.tensor_copy(ulob[:], ulo[:])
    vlob = ppool.tile([128, 30], BF16, tag="vlob")
    nc.vector.tensor_copy(vlob[:], vlo[:])
    unhb = ppool.tile([128, 30], BF16, tag="unhb")
    nc.vector.tensor_copy(unhb[:], unh[:])
    vnhb = ppool.tile([128, 30], BF16, tag="vnhb")
    nc.vector.tensor_copy(vnhb[:], vnh[:])

    # ---- pixel grid rows ----
    it32 = gpool.tile([128, IMG], I32)
    nc.gpsimd.iota(it32[:], pattern=[[1, IMG]], base=0, channel_multiplier=0)
    itf = gpool.tile([128, IMG], F32)
    nc.vector.tensor_copy(itf[:], it32[:])
    ysb = gpool.tile([128, IMG], BF16)   # y_i = (63 - 2i)/64, exact bf16
    nc.vector.tensor_scalar(ysb[:], itf[:], -1.0 / 32.0, 63.0 / 64.0, OP.mult,
                            OP.add)
    xsb = gpool.tile([128, IMG], BF16)   # x_j = (2j - 63)/64
    nc.vector.tensor_scalar(xsb[:], itf[:], 1.0 / 32.0, -63.0 / 64.0, OP.mult,
                            OP.add)


    # xx = [xmat | -xmat] in (s, j, i) layout; built by log2-doubling copies
    xx = gpool.tile([128, 2 * NPIX], BF16)
    xxv = xx[:].rearrange("p (s j i) -> p s j i", s=2, j=IMG)
    nc.vector.tensor_copy(xxv[:, 0, :, 0:1],
                          xsb[:].rearrange("p (j x) -> p j x", x=1))
    w = 1
    while w < IMG:
        nc.vector.tensor_copy(xxv[:, 0, :, w : 2 * w], xxv[:, 0, :, 0:w])
        w *= 2
    nc.vector.tensor_scalar(xx[:, NPIX : 2 * NPIX], xx[:, 0:NPIX], -1.0, None,
                            OP.mult)

    # ---- T planes [128, 1920] bf16 (ft, k, i): t = u*y + v ----
    ub3 = ulob[:].rearrange("p (ft k) -> p ft k", k=3)
    vb3 = vlob[:].rearrange("p (ft k) -> p ft k", k=3)
    un3 = unhb[:].rearrange("p (ft k) -> p ft k", k=3)
    vn3 = vnhb[:].rearrange("p (ft k) -> p ft k", k=3)
    ysbc = ysb[:].rearrange("p (a b i) -> p a b i", a=1, b=1).broadcast_to(
        [128, NTILE, 3, IMG])

    TLO = gpool.tile([128, 1920], BF16)
    TLOv = TLO[:].rearrange("p (ft k i) -> p ft k i", k=3, i=IMG)
    nc.vector.tensor_tensor(TLOv, ub3.unsqueeze(3).broadcast_to(
        [128, NTILE, 3, IMG]), ysbc, OP.mult)
    TLO2 = gpool.tile([128, 1920], BF16)
    TLO2v = TLO2[:].rearrange("p (ft k i) -> p ft k i", k=3, i=IMG)
    nc.vector.tensor_tensor(TLO2v, TLOv, vb3.unsqueeze(3).broadcast_to(
        [128, NTILE, 3, IMG]), OP.add)

    TNH = gpool.tile([128, 1920], BF16)
    TNHv = TNH[:].rearrange("p (ft k i) -> p ft k i", k=3, i=IMG)
    nc.vector.tensor_tensor(TNHv, un3.unsqueeze(3).broadcast_to(
        [128, NTILE, 3, IMG]), ysbc, OP.mult)
    TNH2 = gpool.tile([128, 1920], BF16)
    TNH2v = TNH2[:].rearrange("p (ft k i) -> p ft k i", k=3, i=IMG)
    nc.vector.tensor_tensor(TNH2v, TNHv, vn3.unsqueeze(3).broadcast_to(
        [128, NTILE, 3, IMG]), OP.add)

    # ---- chains -> LH [128, 1280]: cols (s, ft, i); s=0: lo, s=1: -hi ----
    LH = gpool.tile([128, 2 * 640], BF16)
    lo1 = gpool.tile([128, 640], BF16)
    nc.vector.tensor_tensor(lo1[:], TLO2v[:, :, 0, :], TLO2v[:, :, 1, :],
                            OP.max)
    nc.vector.tensor_tensor(
        LH[:, 0:640].rearrange("p (ft i) -> p ft i", i=IMG),
        lo1[:].rearrange("p (ft i) -> p ft i", i=IMG), TLO2v[:, :, 2, :],
        OP.max)
    nh1 = gpool.tile([128, 640], BF16)
    nc.vector.tensor_tensor(nh1[:], TNH2v[:, :, 0, :], TNH2v[:, :, 1, :],
                            OP.max)
    nh2 = gpool.tile([128, 640], BF16)
    nc.vector.tensor_tensor(
        nh2[:].rearrange("p (ft i) -> p ft i", i=IMG),
        nh1[:].rearrange("p (ft i) -> p ft i", i=IMG), TNH2v[:, :, 2, :],
        OP.max)
    # canonicalize empty rows: -hi' = min(-hi, -lo)
    nlo = gpool.tile([128, 640], BF16)
    nc.vector.tensor_scalar(nlo[:], LH[:, 0:640], -1.0, None, OP.mult)
    nc.vector.tensor_tensor(LH[:, 640:1280], nh2[:], nlo[:], OP.min)

    # sliding one-hot for row-targeted PE accumulation
    oh = gpool.tile([128, 16], BF16)
    nc.vector.memset(oh[:], 0.0)
    nc.vector.memset(oh[:, 8:9], 1.0)

    # ---- raster: per face-tile one combined is_ge + 16 accum matmuls ----
    LHv = LH[:].rearrange("p (s ft i) -> p s ft i", s=2, ft=NTILE)
    spool = ctx.enter_context(tc.tile_pool(name="ghp", bufs=3))
    pscnt = ctx.enter_context(tc.tile_pool(name="pcnt", bufs=1, space="PSUM"))
    cnt8 = pscnt.tile([8, 512], F32, tag="cnt8")
    nmm = 0
    NMM = NTILE * 16
    for ft in range(NTILE):
        ghp = spool.tile([128, 2 * NPIX], BF16, tag="ghp")
        lhb = LHv[:, :, ft, :].unsqueeze(2).broadcast_to([128, 2, IMG, IMG])
        nc.vector.tensor_tensor(
            ghp[:].rearrange("p (s j i) -> p s j i", s=2, j=IMG), xxv, lhb,
            OP.is_ge)
        for c in range(16):
            q = c % 8
            nc.tensor.matmul(cnt8[:, :], oh[:, 8 - q : 16 - q],
                             ghp[:, 512 * c : 512 * (c + 1)],
                             start=(nmm == 0), stop=(nmm == NMM - 1))
            nmm += 1

    # ---- threshold: covered iff cnt >= NF + 1 ----
    silb = gpool.tile([8, 512], F32)
    nc.vector.tensor_scalar(silb[:], cnt8[:], NF + 0.5, None, OP.is_gt)
    nc.sync.dma_start(sil_d.ap(), silb[:])


_NC = None


def _get_program():
    global _NC
    if _NC is None:
        nc = bacc.Bacc(
            "TRN2",
            target_bir_lowering=False,
            debug=False,
            enable_asserts=False,
            num_devices=B,
        )
        from contextlib import ExitStack

        with tile.TileContext(nc) as tc:
            with ExitStack() as ctx:
                build_kernel(ctx, tc)
        nc.compile()
        _NC = nc
    return _NC


def _host_layout(vertices, faces):
    """Pure indexing: gather per-face-corner vertices, layout [4, 5120] where
    column n = ft*512 + k*128 + p holds corner k of face ft*128+p; row 3 = 1."""
    faces4 = np.concatenate([faces, faces[:, :1]], axis=1)  # [1280, 4]
    vidx = faces4.reshape(NTILE, 128, 4).transpose(0, 2, 1).reshape(-1)
    out = []
    ones = np.ones((1, NCOL), dtype=np.float32)
    for b in range(B):
        vg = vertices[b][vidx]  # [5120, 3]
        out.append(np.ascontiguousarray(
            np.concatenate([vg.T.astype(np.float32), ones], axis=0)))
    return out


def kernel(vertices, viewpoints, faces, img_size):
    vertices = np.asarray(vertices, dtype=np.float32)
    viewpoints = np.asarray(viewpoints, dtype=np.float32)
    faces = np.asarray(faces, dtype=np.int32)
    assert int(img_size) == IMG and vertices.shape == (B, V, 3)

    nc = _get_program()
    vgts = _host_layout(vertices, faces)
    in_maps = [
        {"vgt4": vgts[b], "eye": np.ascontiguousarray(viewpoints[b])}
        for b in range(B)
    ]
    res = run_bass_kernel_spmd(nc, in_maps, core_ids=list(range(B)))
    # device pixel order is (j, i): transpose back to raster (i, j)
    sil = np.stack([
        res.results[b]["sil"].reshape(IMG, IMG).T for b in range(B)
    ])
    return sil.reshape(B, 1, IMG, IMG).astype(np.float32)


if __name__ == "__main__":
    rng = np.random.default_rng(0)
    verts = rng.standard_normal((B, V, 3), dtype=np.float32) * 0.5
    vps = rng.standard_normal((B, 3), dtype=np.float32)
    fcs = rng.integers(0, V, (NF, 3), dtype=np.int32)
    out = kernel(verts, vps, fcs, IMG)
    print(out.shape, out.sum())


# revision 19
# speedup vs baseline: 2.5818x; 1.1061x over previous
"""Trainium2 Bass kernel for nn_Mesh_Renderer: silhouette via scanline intervals.

Data-parallel over batch (core b renders view b). Host work is layout only
(gather vertices[faces], constant grid/basis tables, transpose the returned
image). All input-dependent math on device.

Device algorithm (per core):
  1. look_at camera basis from eye; projection folded as [w;1]^T @ [R^T; -R@eye]
     (40 K=4 f32 matmuls), perspective divide -> per-corner (xn, yn) [128, 40].
  2. Edge coefficients per (face, edge): e = A x + B y + C. For each pixel row
     y_i the face coverage in x is an interval [lo, hi]:
       t_k(i) = -(B_k y_i + C_k)/A_k ; edge k bounds from below iff
       sign(2*area)*A_k > 0. lo = max over lower-edges, -hi = max over upper
       (negated). Invisible/degenerate faces forced to a contributes-nothing
       interval via +-BIG offsets folded into the per-edge (u, v) small tiles;
       empty rows canonicalized with -hi' = min(-hi, -lo) (point interval).
     The t-planes t = u*y + v are evaluated by PE against a constant
     block-diagonal basis (tbasis), with (u, v) PE-transposed into lhsT.
  3. Raster: count(i,j) = sum_f([x_j >= lo] + [x_j <= hi]) = F + #covering.
     One DVE is_ge over [128, 8192] per 128-face tile computes both compares
     ([x | -x] vs [lo | -hi] broadcast over j); PE ones-matmuls accumulate
     over faces into PSUM cnt8 [8, 512] (sliding-onehot lhsT selects the row).
  4. silhouette = cnt >= F+1; DMA out; host transposes (j,i)->(i,j).
"""

import sys

if "/opt/trn_rl_repo" not in sys.path:
    sys.path.insert(0, "/opt/trn_rl_repo")

import ml_dtypes
import numpy as np

import concourse.bacc as bacc
import concourse.tile as tile
from concourse import mybir
from concourse.bass_utils import run_bass_kernel_spmd

F32 = mybir.dt.float32
BF16 = mybir.dt.bfloat16
I32 = mybir.dt.int32
OP = mybir.AluOpType
AF = mybir.ActivationFunctionType

B, V, NF, IMG = 8, 642, 1280, 64
NPIX = IMG * IMG          # 4096
NTILE = NF // 128         # 10 face tiles
NCOL = NF * 4             # 5120 gathered corners (a, b, c, a)
EPS = 1e-8
BIG = 1.0e30
TAN_T = float(np.tan(np.deg2rad(np.float32(15.0)).astype(np.float32)))


def _normalize3(nc, pool, v, name):
    """v [1,3] f32 -> v * rsqrt(sum v^2); margins cover the eps difference."""
    sq = pool.tile([1, 3], F32, name=f"{name}_sq")
    nc.vector.tensor_tensor(sq[:], v[:], v[:], OP.mult)
    s = pool.tile([1, 1], F32, name=f"{name}_s")
    nc.vector.tensor_reduce(s[:], sq[:], mybir.AxisListType.X, OP.add)
    n = pool.tile([1, 1], F32, name=f"{name}_n")
    nc.scalar.activation(n[:], s[:], AF.Sqrt)
    r = pool.tile([1, 1], F32, name=f"{name}_r")
    nc.vector.reciprocal(r[:], n[:])
    out = pool.tile([1, 3], F32, name=f"{name}_out")
    nc.vector.tensor_scalar(out[:], v[:], r[:], None, OP.mult)
    return out


def _cross3(nc, pool, a, b, name):
    a2 = pool.tile([1, 6], F32, name=f"{name}_a2")
    nc.vector.tensor_copy(a2[:, 0:3], a[:])
    nc.vector.tensor_copy(a2[:, 3:6], a[:])
    b2 = pool.tile([1, 6], F32, name=f"{name}_b2")
    nc.vector.tensor_copy(b2[:, 0:3], b[:])
    nc.vector.tensor_copy(b2[:, 3:6], b[:])
    m1 = pool.tile([1, 3], F32, name=f"{name}_m1")
    nc.vector.tensor_tensor(m1[:], a2[:, 1:4], b2[:, 2:5], OP.mult)
    m2 = pool.tile([1, 3], F32, name=f"{name}_m2")
    nc.vector.tensor_tensor(m2[:], a2[:, 2:5], b2[:, 1:4], OP.mult)
    out = pool.tile([1, 3], F32, name=f"{name}_out")
    nc.vector.tensor_tensor(out[:], m1[:], m2[:], OP.subtract)
    return out


def build_kernel(ctx, tc):
    nc = tc.nc
    vgt_d = nc.dram_tensor("vgt16", [16, NF], F32, kind="ExternalInput")
    eye_d = nc.dram_tensor("eye", [3], F32, kind="ExternalInput")
    xg_d = nc.dram_tensor("xgrid", [128, 2 * NPIX], BF16, kind="ExternalInput")
    tb_d = nc.dram_tensor("tbasis", [60, 1920], BF16, kind="ExternalInput")
    sil_d = nc.dram_tensor("sil", [NPIX], F32, kind="ExternalOutput")

    cpool = ctx.enter_context(tc.tile_pool(name="cam", bufs=1))
    ppool = ctx.enter_context(tc.tile_pool(name="proj", bufs=1))
    gpool = ctx.enter_context(tc.tile_pool(name="grid", bufs=1))

    # ---- input DMAs ----
    eyeR = cpool.tile([1, 3], F32)
    nc.sync.dma_start(eyeR[:], eye_d.ap())
    vgt = gpool.tile([16, NF], F32)
    nc.sync.dma_start(vgt[:], vgt_d.ap())
    xx = gpool.tile([128, 2 * NPIX], BF16)
    nc.sync.dma_start(xx[:], xg_d.ap())
    tb = gpool.tile([60, 1920], BF16)
    nc.sync.dma_start(tb[:], tb_d.ap())
    xxv = xx[:].rearrange("p (s j i) -> p s j i", s=2, j=IMG)

    # identity for PE transposes (iotas on Pool, rest tiny)
    iop = gpool.tile([128, 1], I32)
    nc.gpsimd.iota(iop[:], pattern=[[1, 1]], base=0, channel_multiplier=1)
    iopf = gpool.tile([128, 1], F32)
    nc.vector.tensor_copy(iopf[:], iop[:])
    iof = gpool.tile([128, 128], I32)
    nc.gpsimd.iota(iof[:], pattern=[[1, 128]], base=0, channel_multiplier=0)
    ioff = gpool.tile([128, 128], F32)
    nc.vector.tensor_copy(ioff[:], iof[:])
    idm = gpool.tile([128, 128], F32)
    nc.vector.tensor_scalar(idm[:], ioff[:], iopf[:], None, OP.is_equal)

    # sliding one-hot for row-targeted PE accumulation
    oh = gpool.tile([128, 16], BF16)
    nc.gpsimd.memset(oh[:], 0.0)
    nc.gpsimd.memset(oh[:, 8:9], 1.0)

    # ---- camera basis (partition 0, tiny tiles) ----
    # x_ax dir = cross(up, z) = cross(up, -eye) up to positive scale, so the
    # x/y chain runs off -eye directly; z-normalize is off the critical path.
    nege = cpool.tile([1, 3], F32)
    nc.vector.tensor_scalar(nege[:], eyeR[:], -1.0, None, OP.mult)
    xr = cpool.tile([1, 3], F32)
    nc.vector.memset(xr[:], 0.0)
    nc.vector.tensor_copy(xr[:, 0:1], nege[:, 2:3])
    nc.vector.tensor_scalar(xr[:, 2:3], nege[:, 0:1], -1.0, None, OP.mult)
    x_ax = _normalize3(nc, cpool, xr, "nx")
    yr = _cross3(nc, cpool, nege, x_ax, "cy")
    y_ax = _normalize3(nc, cpool, yr, "ny")
    z_ax = _normalize3(nc, cpool, nege, "nz")

    # rt16 = 4 diagonal copies of rt4 = [R^T; -(eye^T @ R^T)] (one per corner),
    # staged row-major on partition 0 and reshaped by a single DMA.
    # stage[0, r*12 + c]; block k: rows 4k+d' cols 3k+d hold R[d, d'] and row
    # 4k+3 holds -Reye[d].
    rtT9 = cpool.tile([1, 9], F32)   # rtT9[0, 3*d' + d] = axis_d[d']
    for d, axis in enumerate([x_ax, y_ax, z_ax]):
        nc.vector.tensor_copy(
            rtT9[:].rearrange("p (dp d) -> p dp d", d=3)[:, :, d], axis[:])
    # -Reye[d] = -sum_dp eye[dp] * R^T[dp, d] via elementwise + X-reduce
    el = cpool.tile([1, 9], F32)   # (d, dp) layout
    nc.vector.tensor_tensor(
        el[:].rearrange("p (d dp) -> p d dp", dp=3),
        rtT9[:].rearrange("p (dp d) -> p d dp", d=3),
        eyeR[:].unsqueeze(1).broadcast_to([1, 3, 3]), OP.mult)
    nreye0 = cpool.tile([1, 3], F32)
    nc.vector.tensor_reduce(nreye0[:], el[:].rearrange(
        "p (d dp) -> p d dp", dp=3), mybir.AxisListType.X, OP.add)
    nreye = cpool.tile([1, 3], F32)
    nc.vector.tensor_scalar(nreye[:], nreye0[:], -1.0, None, OP.mult)
    stage = cpool.tile([1, 192], F32)
    nc.vector.memset(stage[:], 0.0)
    rtv = rtT9[:].rearrange("p (dp d) -> p dp d", d=3)
    for k in range(4):
        base = 51 * k  # block k: coord rows at 51k + 12d' + d, ones at +36+d
        nc.vector.tensor_copy(
            stage[:, base : base + 36].rearrange(
                "p (dp c) -> p dp c", c=12)[:, :, 0:3], rtv)
        nc.vector.tensor_copy(stage[:, base + 36 : base + 39], nreye[:])
    rt16 = cpool.tile([16, 12], F32)
    nc.sync.dma_start(rt16[:], stage[:])

    # ---- projection: vca[p, (ft, k, d)] = [w;1]^T @ rt4 per corner ----
    vca = ppool.tile([128, 120], F32)
    with tc.tile_pool(name="pvc", bufs=1, space="PSUM") as psvc:
        vcp = psvc.tile([128, 120], F32)
        for ft in range(NTILE):
            nc.tensor.matmul(
                vcp[:, 12 * ft : 12 * (ft + 1)],
                vgt[:, 128 * ft : 128 * (ft + 1)],
                rt16[:],
                start=True,
                stop=True,
            )
        nc.vector.tensor_copy(vca[:], vcp[:])

    vcav = vca[:].rearrange("p (c d) -> p c d", d=3)
    vx, vy, vz = vcav[:, :, 0], vcav[:, :, 1], vcav[:, :, 2]

    # perspective divide (raw reciprocal; interval margins tolerate ~3e-3)
    dn = ppool.tile([128, 40], F32)
    nc.vector.tensor_scalar(dn[:], vz, TAN_T, EPS, OP.mult, OP.add)
    rc = ppool.tile([128, 40], F32)
    nc.vector.reciprocal(rc[:], dn[:])
    xn = ppool.tile([128, 40], F32)
    nc.vector.tensor_tensor(xn[:], vx, rc[:], OP.mult)
    yn = ppool.tile([128, 40], F32)
    nc.vector.tensor_tensor(yn[:], vy, rc[:], OP.mult)

    # visibility: all corner z > 0 (on Pool)
    vz4 = vca[:].rearrange("p (ft k d) -> p ft k d", k=4, d=3)
    mz1 = ppool.tile([128, 10], F32)
    nc.vector.tensor_tensor(mz1[:], vz4[:, :, 0, 2], vz4[:, :, 1, 2], OP.min)
    mz = ppool.tile([128, 10], F32)
    nc.vector.tensor_tensor(mz[:], mz1[:], vz4[:, :, 2, 2], OP.min)
    vg = ppool.tile([128, 10], F32)
    nc.vector.tensor_scalar(vg[:], mz[:], 0.0, None, OP.is_gt)

    # ---- edge coefficients [128, 30] in (ft, k) layout ----
    xn4 = xn[:].rearrange("p (ft k) -> p ft k", k=4)
    yn4 = yn[:].rearrange("p (ft k) -> p ft k", k=4)
    xk, xk1 = xn4[:, :, 0:3], xn4[:, :, 1:4]
    yk, yk1 = yn4[:, :, 0:3], yn4[:, :, 1:4]

    def t30(name, eng=None):
        return ppool.tile([128, 30], F32, name=name, tag=name)

    A = t30("A")
    Av = A[:].rearrange("p (ft k) -> p ft k", k=3)
    nc.vector.tensor_tensor(Av, yk, yk1, OP.subtract)
    Bc = t30("Bc")
    Bv = Bc[:].rearrange("p (ft k) -> p ft k", k=3)
    nc.vector.tensor_tensor(Bv, xk1, xk, OP.subtract)
    p1 = t30("p1")
    nc.gpsimd.tensor_tensor(p1[:].rearrange("p (ft k) -> p ft k", k=3), xk,
                            yk1, OP.mult)
    p2 = t30("p2")
    nc.gpsimd.tensor_tensor(p2[:].rearrange("p (ft k) -> p ft k", k=3), yk,
                            xk1, OP.mult)
    C = t30("C")
    nc.gpsimd.tensor_tensor(C[:], p1[:], p2[:], OP.subtract)

    Cv = C[:].rearrange("p (ft k) -> p ft k", k=3)
    S1 = ppool.tile([128, 10], F32, name="S1")
    nc.gpsimd.tensor_tensor(S1[:], Cv[:, :, 0], Cv[:, :, 1], OP.add)
    S = ppool.tile([128, 10], F32, name="S")
    nc.gpsimd.tensor_tensor(S[:], S1[:], Cv[:, :, 2], OP.add)

    # masks (Pool side-chain)
    w = t30("w")
    nc.gpsimd.tensor_tensor(w[:].rearrange("p (ft k) -> p ft k", k=3), Av,
                            S[:].unsqueeze(2).broadcast_to([128, 10, 3]),
                            OP.mult)
    mpos = t30("mpos")
    nc.vector.tensor_scalar(mpos[:], w[:], 0.0, None, OP.is_gt)
    mneg = t30("mneg")
    nc.vector.tensor_scalar(mneg[:], w[:], 0.0, None, OP.is_lt)
    offlo = t30("offlo")
    nc.vector.tensor_scalar(offlo[:], mpos[:], BIG, -BIG, OP.mult, OP.add)
    offnh = t30("offnh")
    nc.vector.tensor_scalar(offnh[:], mneg[:], BIG, -BIG, OP.mult, OP.add)
    mnegN = t30("mnegN")
    nc.vector.tensor_scalar(mnegN[:], mneg[:], -1.0, None, OP.mult)

    sne = ppool.tile([128, 10], F32, name="sne")
    nc.vector.tensor_scalar(sne[:], S[:], 0.0, None, OP.not_equal)
    visq = ppool.tile([128, 10], F32, name="visq")
    nc.gpsimd.tensor_tensor(visq[:], vg[:], sne[:], OP.mult)
    ivq = ppool.tile([128, 10], F32, name="ivq")
    nc.vector.tensor_scalar(ivq[:], visq[:], -2.0 * BIG, 2.0 * BIG, OP.mult,
                            OP.add)
    ivqN = ppool.tile([128, 10], F32, name="ivqN")
    nc.vector.tensor_scalar(ivqN[:], visq[:], 2.0 * BIG, -2.0 * BIG, OP.mult,
                            OP.add)

    # reciprocal side (DVE)
    iseq = t30("iseq")
    nc.vector.tensor_scalar(iseq[:], A[:], 0.0, None, OP.is_equal)
    Asafe = t30("Asafe")
    nc.vector.tensor_tensor(Asafe[:], A[:], iseq[:], OP.add)
    r0 = t30("r0")
    nc.vector.reciprocal(r0[:], Asafe[:])
    nr = t30("nr")
    nc.vector.tensor_scalar(nr[:], r0[:], -1.0, None, OP.mult)
    u = t30("u")
    nc.vector.tensor_tensor(u[:], Bc[:], nr[:], OP.mult)
    v = t30("v")
    nc.vector.tensor_tensor(v[:], C[:], nr[:], OP.mult)

    # (u, v) -> interleaved lhsT staging tiles [128, 60]: col 2m = u_m, 2m+1 = v_m
    uvlo = ppool.tile([128, 60], F32, name="uvlo")
    uvlov = uvlo[:].rearrange("p (m two) -> p m two", two=2)
    uvnh = ppool.tile([128, 60], F32, name="uvnh")
    uvnhv = uvnh[:].rearrange("p (m two) -> p m two", two=2)

    # lower side: ulo = u*mpos ; vlo = v*mpos - BIG*(1-mpos) + ivq
    nc.vector.tensor_tensor(uvlov[:, :, 0], u[:], mpos[:], OP.mult)
    vlo1 = t30("vlo1")
    nc.vector.tensor_tensor(vlo1[:], v[:], mpos[:], OP.mult)
    vlo2 = t30("vlo2")
    nc.vector.tensor_tensor(vlo2[:], vlo1[:], offlo[:], OP.add)
    nc.vector.tensor_tensor(
        uvlov[:, :, 1].rearrange("p (ft k) -> p ft k", k=3),
        vlo2[:].rearrange("p (ft k) -> p ft k", k=3),
        ivq[:].unsqueeze(2).broadcast_to([128, 10, 3]), OP.add)

    # negated upper side: unh = -u*mneg ; vnh = -v*mneg - BIG*(1-mneg) - ivq
    nc.vector.tensor_tensor(uvnhv[:, :, 0], u[:], mnegN[:], OP.mult)
    vnh1 = t30("vnh1")
    nc.vector.tensor_tensor(vnh1[:], v[:], mnegN[:], OP.mult)
    vnh2 = t30("vnh2")
    nc.vector.tensor_tensor(vnh2[:], vnh1[:], offnh[:], OP.add)
    nc.vector.tensor_tensor(
        uvnhv[:, :, 1].rearrange("p (ft k) -> p ft k", k=3),
        vnh2[:].rearrange("p (ft k) -> p ft k", k=3),
        ivqN[:].unsqueeze(2).broadcast_to([128, 10, 3]), OP.add)

    # ---- T planes via PE: transpose (u,v), matmul against constant basis ----
    TLOs = gpool.tile([128, 1920], BF16)
    TNHs = gpool.tile([128, 1920], BF16)
    with tc.tile_pool(name="ptr", bufs=2, space="PSUM") as ptr:
        uvloT = ptr.tile([60, 128], F32, tag="uvT")
        nc.tensor.transpose(uvloT[:], uvlo[:], idm[:])
        uvloB = gpool.tile([60, 128], BF16)
        nc.scalar.activation(uvloB[:], uvloT[:], AF.Copy)
        uvnhT = ptr.tile([60, 128], F32, tag="uvT")
        nc.tensor.transpose(uvnhT[:], uvnh[:], idm[:])
        uvnhB = gpool.tile([60, 128], BF16)
        nc.scalar.activation(uvnhB[:], uvnhT[:], AF.Copy)
    with tc.tile_pool(name="ptp", bufs=2, space="PSUM") as ptp:
        TLOp = ptp.tile([128, 1920], F32, tag="tp")
        for q in range(4):
            nc.tensor.matmul(TLOp[:, 480 * q : 480 * (q + 1)], uvloB[:],
                             tb[:, 480 * q : 480 * (q + 1)], start=True,
                             stop=True)
        nc.scalar.activation(TLOs[:], TLOp[:], AF.Copy)
        TNHp = ptp.tile([128, 1920], F32, tag="tp")
        for q in range(4):
            nc.tensor.matmul(TNHp[:, 480 * q : 480 * (q + 1)], uvnhB[:],
                             tb[:, 480 * q : 480 * (q + 1)], start=True,
                             stop=True)
        nc.scalar.activation(TNHs[:], TNHp[:], AF.Copy)

    # ---- chains -> LH [128, 1280]: cols (s, ft, i); s=0: lo, s=1: -hi ----
    TLOv = TLOs[:].rearrange("p (ft k i) -> p ft k i", k=3, i=IMG)
    TNHv = TNHs[:].rearrange("p (ft k i) -> p ft k i", k=3, i=IMG)
    LH = gpool.tile([128, 2 * 640], BF16)
    lo1 = gpool.tile([128, 640], BF16)
    nc.vector.tensor_tensor(lo1[:], TLOv[:, :, 0, :], TLOv[:, :, 1, :], OP.max)
    nc.vector.tensor_tensor(
        LH[:, 0:640].rearrange("p (ft i) -> p ft i", i=IMG),
        lo1[:].rearrange("p (ft i) -> p ft i", i=IMG), TLOv[:, :, 2, :],
        OP.max)
    nh1 = gpool.tile([128, 640], BF16)
    nc.vector.tensor_tensor(nh1[:], TNHv[:, :, 0, :], TNHv[:, :, 1, :], OP.max)
    nh2 = gpool.tile([128, 640], BF16)
    nc.vector.tensor_tensor(
        nh2[:].rearrange("p (ft i) -> p ft i", i=IMG),
        nh1[:].rearrange("p (ft i) -> p ft i", i=IMG), TNHv[:, :, 2, :],
        OP.max)
    # canonicalize empty rows: -hi' = min(-hi, -lo)
    nlo = gpool.tile([128, 640], BF16)
    nc.vector.tensor_scalar(nlo[:], LH[:, 0:640], -1.0, None, OP.mult)
    nc.vector.tensor_tensor(LH[:, 640:1280], nh2[:], nlo[:], OP.min)

    # ---- raster: per face-tile one combined is_ge + 16 accum matmuls ----
    # Junk "warmer" matmuls keep the PE p-state ramped: a pre-raster burst
    # while the first compare runs, plus a couple per face-tile to bridge the
    # compare/accumulate rate gap without the engine ever going idle.
    LHv = LH[:].rearrange("p (s ft i) -> p s ft i", s=2, ft=NTILE)
    spool = ctx.enter_context(tc.tile_pool(name="ghp", bufs=3))
    pscnt = ctx.enter_context(tc.tile_pool(name="pcnt", bufs=1, space="PSUM"))
    pwarm = ctx.enter_context(tc.tile_pool(name="pwarm", bufs=1, space="PSUM"))
    cnt8 = pscnt.tile([8, 512], F32, tag="cnt8")
    wps = pwarm.tile([128, 480], F32, tag="wps")

    def warm(n):
        for wq in range(n):
            nc.tensor.matmul(wps[:], uvloB[:], tb[:, 0:480], start=True,
                             stop=True)

    warm(10)
    nmm = 0
    NMM = NTILE * 16
    for ft in range(NTILE):
        ghp = spool.tile([128, 2 * NPIX], BF16, tag="ghp")
        lhb = LHv[:, :, ft, :].unsqueeze(2).broadcast_to([128, 2, IMG, IMG])
        nc.vector.tensor_tensor(
            ghp[:].rearrange("p (s j i) -> p s j i", s=2, j=IMG), xxv, lhb,
            OP.is_ge)
        for c in range(16):
            q = c % 8
            nc.tensor.matmul(cnt8[:, :], oh[:, 8 - q : 16 - q],
                             ghp[:, 512 * c : 512 * (c + 1)],
                             start=(nmm == 0), stop=(nmm == NMM - 1))
            nmm += 1
        if ft < NTILE - 1:
            warm(2)

    # ---- threshold: covered iff cnt >= NF + 1 ----
    silb = gpool.tile([8, 512], F32)
    nc.vector.tensor_scalar(silb[:], cnt8[:], NF + 0.5, None, OP.is_gt)
    nc.sync.dma_start(sil_d.ap(), silb[:])


_NC = None


def _get_program():
    global _NC
    if _NC is None:
        nc = bacc.Bacc(
            "TRN2",
            target_bir_lowering=False,
            debug=False,
            enable_asserts=False,
            num_devices=B,
        )
        from contextlib import ExitStack

        with tile.TileContext(nc) as tc:
            with ExitStack() as ctx:
                build_kernel(ctx, tc)
        nc.compile()
        _NC = nc
    return _NC


def _consts():
    """Input-independent constant tables (pixel grid, t-plane basis)."""
    j = np.arange(IMG, dtype=np.float32)
    xs = (2.0 * j - 63.0) / 64.0                      # exact in bf16
    ys = (63.0 - 2.0 * j) / 64.0
    xg = np.empty((2, IMG, IMG), dtype=np.float32)
    xg[0] = xs[:, None]
    xg[1] = -xs[:, None]
    xgrid = np.broadcast_to(xg.reshape(1, 2 * NPIX), (128, 2 * NPIX))
    xgrid = np.ascontiguousarray(xgrid).astype(ml_dtypes.bfloat16)
    tb = np.zeros((60, 1920), dtype=np.float32)
    for m in range(30):
        tb[2 * m, m * 64 : (m + 1) * 64] = ys
        tb[2 * m + 1, m * 64 : (m + 1) * 64] = 1.0
    tbasis = tb.astype(ml_dtypes.bfloat16)
    return xgrid, tbasis


def _host_layout(vertices, faces):
    """Pure indexing: gather per-face-corner vertices into [16, 1280] where
    row 4k+d / column ft*128+p holds coord d (d=3: 1.0) of corner k of face
    ft*128+p; corners are (a, b, c, a)."""
    faces4 = np.concatenate([faces, faces[:, :1]], axis=1)  # [1280, 4]
    out = []
    for b in range(B):
        vg = vertices[b][faces4]                      # [1280, 4, 3]
        vg4 = np.concatenate(
            [vg, np.ones((NF, 4, 1), dtype=np.float32)], axis=2)  # [1280,4,4]
        out.append(np.ascontiguousarray(
            vg4.transpose(1, 2, 0).reshape(16, NF).astype(np.float32)))
    return out


def kernel(vertices, viewpoints, faces, img_size):
    vertices = np.asarray(vertices, dtype=np.float32)
    viewpoints = np.asarray(viewpoints, dtype=np.float32)
    faces = np.asarray(faces, dtype=np.int32)
    assert int(img_size) == IMG and vertices.shape == (B, V, 3)

    nc = _get_program()
    vgts = _host_layout(vertices, faces)
    xgrid, tbasis = _consts()
    in_maps = [
        {"vgt16": vgts[b], "eye": np.ascontiguousarray(viewpoints[b]),
         "xgrid": xgrid, "tbasis": tbasis}
        for b in range(B)
    ]
    res = run_bass_kernel_spmd(nc, in_maps, core_ids=list(range(B)))
    # device pixel order is (j, i): transpose back to raster (i, j)
    sil = np.stack([
        res.results[b]["sil"].reshape(IMG, IMG).T for b in range(B)
    ])
    return sil.reshape(B, 1, IMG, IMG).astype(np.float32)


if __name__ == "__main__":
    rng = np.random.default_rng(0)
    verts = rng.standard_normal((B, V, 3), dtype=np.float32) * 0.5
    vps = rng.standard_normal((B, 3), dtype=np.float32)
    fcs = rng.integers(0, V, (NF, 3), dtype=np.int32)
    out = kernel(verts, vps, fcs, IMG)
    print(out.shape, out.sum())


# revision 25
# speedup vs baseline: 2.6398x; 1.0224x over previous
"""Trainium2 Bass kernel for nn_Mesh_Renderer: silhouette via scanline intervals.

Data-parallel over batch (core b renders view b). Host work is layout only
(gather vertices[faces], constant grid/basis tables, transpose the returned
image). All input-dependent math on device.

Device algorithm (per core):
  1. look_at camera basis from eye; projection folded as [w;1]^T @ [R^T; -R@eye]
     (40 K=4 f32 matmuls), perspective divide -> per-corner (xn, yn) [128, 40].
  2. Edge coefficients per (face, edge): e = A x + B y + C. For each pixel row
     y_i the face coverage in x is an interval [lo, hi]:
       t_k(i) = -(B_k y_i + C_k)/A_k ; edge k bounds from below iff
       sign(2*area)*A_k > 0. lo = max over lower-edges, -hi = max over upper
       (negated). Invisible/degenerate faces forced to a contributes-nothing
       interval via +-BIG offsets folded into the per-edge (u, v) small tiles;
       empty rows canonicalized with -hi' = min(-hi, -lo) (point interval).
     The t-planes t = u*y + v are evaluated by PE against a constant
     block-diagonal basis (tbasis), with (u, v) PE-transposed into lhsT.
  3. Raster: count(i,j) = sum_f([x_j >= lo] + [x_j <= hi]) = F + #covering.
     One DVE is_ge over [128, 8192] per 128-face tile computes both compares
     ([x | -x] vs [lo | -hi] broadcast over j); PE ones-matmuls accumulate
     over faces into PSUM cnt8 [8, 512] (sliding-onehot lhsT selects the row).
  4. silhouette = cnt >= F+1; DMA out; host transposes (j,i)->(i,j).
"""

import sys

if "/opt/trn_rl_repo" not in sys.path:
    sys.path.insert(0, "/opt/trn_rl_repo")

import ml_dtypes
import numpy as np

import concourse.bacc as bacc
import concourse.tile as tile
from concourse import mybir
from concourse.bass_utils import run_bass_kernel_spmd

F32 = mybir.dt.float32
BF16 = mybir.dt.bfloat16
I32 = mybir.dt.int32
OP = mybir.AluOpType
AF = mybir.ActivationFunctionType

B, V, NF, IMG = 8, 642, 1280, 64
NPIX = IMG * IMG          # 4096
NTILE = NF // 128         # 10 face tiles
NCOL = NF * 4             # 5120 gathered corners (a, b, c, a)
EPS = 1e-8
BIG = 1.0e30
TAN_T = float(np.tan(np.deg2rad(np.float32(15.0)).astype(np.float32)))


def _normalize3(nc, pool, v, name):
    """v [1,3] f32 -> v * rsqrt(sum v^2); margins cover the eps difference."""
    sq = pool.tile([1, 3], F32, name=f"{name}_sq")
    nc.vector.tensor_tensor(sq[:], v[:], v[:], OP.mult)
    s = pool.tile([1, 1], F32, name=f"{name}_s")
    nc.vector.tensor_reduce(s[:], sq[:], mybir.AxisListType.X, OP.add)
    n = pool.tile([1, 1], F32, name=f"{name}_n")
    nc.scalar.activation(n[:], s[:], AF.Sqrt)
    r = pool.tile([1, 1], F32, name=f"{name}_r")
    nc.vector.reciprocal(r[:], n[:])
    out = pool.tile([1, 3], F32, name=f"{name}_out")
    nc.vector.tensor_scalar(out[:], v[:], r[:], None, OP.mult)
    return out


def _cross3(nc, pool, a, b, name):
    a2 = pool.tile([1, 6], F32, name=f"{name}_a2")
    nc.vector.tensor_copy(a2[:, 0:3], a[:])
    nc.vector.tensor_copy(a2[:, 3:6], a[:])
    b2 = pool.tile([1, 6], F32, name=f"{name}_b2")
    nc.vector.tensor_copy(b2[:, 0:3], b[:])
    nc.vector.tensor_copy(b2[:, 3:6], b[:])
    m1 = pool.tile([1, 3], F32, name=f"{name}_m1")
    nc.vector.tensor_tensor(m1[:], a2[:, 1:4], b2[:, 2:5], OP.mult)
    m2 = pool.tile([1, 3], F32, name=f"{name}_m2")
    nc.vector.tensor_tensor(m2[:], a2[:, 2:5], b2[:, 1:4], OP.mult)
    out = pool.tile([1, 3], F32, name=f"{name}_out")
    nc.vector.tensor_tensor(out[:], m1[:], m2[:], OP.subtract)
    return out


def build_kernel(ctx, tc):
    nc = tc.nc
    vgt_d = nc.dram_tensor("vgt16", [16, NF], F32, kind="ExternalInput")
    eye_d = nc.dram_tensor("eye", [3], F32, kind="ExternalInput")
    xg_d = nc.dram_tensor("xgrid", [128, 2 * NPIX], BF16, kind="ExternalInput")
    tb_d = nc.dram_tensor("tbasis", [60, 1920], BF16, kind="ExternalInput")
    sil_d = nc.dram_tensor("sil", [NPIX], F32, kind="ExternalOutput")

    cpool = ctx.enter_context(tc.tile_pool(name="cam", bufs=1))
    ppool = ctx.enter_context(tc.tile_pool(name="proj", bufs=1))
    gpool = ctx.enter_context(tc.tile_pool(name="grid", bufs=1))

    # ---- input DMAs ----
    eyeR = cpool.tile([1, 3], F32)
    nc.sync.dma_start(eyeR[:], eye_d.ap())
    vgt = gpool.tile([16, NF], F32)
    nc.sync.dma_start(vgt[:], vgt_d.ap())
    tb = gpool.tile([60, 1920], BF16)
    nc.sync.dma_start(tb[:], tb_d.ap())
    xx = gpool.tile([128, 2 * NPIX], BF16)
    xxv = xx[:].rearrange("p (s j i) -> p s j i", s=2, j=IMG)

    # identity for PE transposes (iotas on Pool, rest tiny)
    iop = gpool.tile([128, 1], I32)
    nc.gpsimd.iota(iop[:], pattern=[[1, 1]], base=0, channel_multiplier=1)
    iopf = gpool.tile([128, 1], F32)
    nc.vector.tensor_copy(iopf[:], iop[:])
    iof = gpool.tile([128, 128], I32)
    nc.gpsimd.iota(iof[:], pattern=[[1, 128]], base=0, channel_multiplier=0)
    ioff = gpool.tile([128, 128], F32)
    nc.vector.tensor_copy(ioff[:], iof[:])
    idm = gpool.tile([128, 128], F32)
    nc.vector.tensor_scalar(idm[:], ioff[:], iopf[:], None, OP.is_equal)

    # sliding one-hot for row-targeted PE accumulation
    oh = gpool.tile([128, 16], BF16)
    nc.gpsimd.memset(oh[:], 0.0)
    nc.gpsimd.memset(oh[:, 8:9], 1.0)

    # ---- camera basis (partition 0, tiny tiles) ----
    # x_ax dir = cross(up, z) = cross(up, -eye) up to positive scale, so the
    # x/y chain runs off -eye directly; z-normalize is off the critical path.
    nege = cpool.tile([1, 3], F32)
    nc.vector.tensor_scalar(nege[:], eyeR[:], -1.0, None, OP.mult)
    xr = cpool.tile([1, 3], F32)
    nc.vector.memset(xr[:], 0.0)
    nc.vector.tensor_copy(xr[:, 0:1], nege[:, 2:3])
    nc.vector.tensor_scalar(xr[:, 2:3], nege[:, 0:1], -1.0, None, OP.mult)
    x_ax = _normalize3(nc, cpool, xr, "nx")
    yr = _cross3(nc, cpool, nege, x_ax, "cy")
    y_ax = _normalize3(nc, cpool, yr, "ny")
    z_ax = _normalize3(nc, cpool, nege, "nz")

    # rt16 = 4 diagonal copies of rt4 = [R^T; -(eye^T @ R^T)] (one per corner),
    # staged row-major on partition 0 and reshaped by a single DMA.
    # stage[0, r*12 + c]; block k: rows 4k+d' cols 3k+d hold R[d, d'] and row
    # 4k+3 holds -Reye[d].
    rtT9 = cpool.tile([1, 9], F32)   # rtT9[0, 3*d' + d] = axis_d[d']
    for d, axis in enumerate([x_ax, y_ax, z_ax]):
        nc.vector.tensor_copy(
            rtT9[:].rearrange("p (dp d) -> p dp d", d=3)[:, :, d], axis[:])
    # -Reye[d] = -sum_dp eye[dp] * R^T[dp, d] via elementwise + X-reduce
    el = cpool.tile([1, 9], F32)   # (d, dp) layout
    nc.vector.tensor_tensor(
        el[:].rearrange("p (d dp) -> p d dp", dp=3),
        rtT9[:].rearrange("p (dp d) -> p d dp", d=3),
        eyeR[:].unsqueeze(1).broadcast_to([1, 3, 3]), OP.mult)
    nreye0 = cpool.tile([1, 3], F32)
    nc.vector.tensor_reduce(nreye0[:], el[:].rearrange(
        "p (d dp) -> p d dp", dp=3), mybir.AxisListType.X, OP.add)
    nreye = cpool.tile([1, 3], F32)
    nc.vector.tensor_scalar(nreye[:], nreye0[:], -1.0, None, OP.mult)
    stage = cpool.tile([1, 192], F32)
    nc.vector.memset(stage[:], 0.0)
    rtv = rtT9[:].rearrange("p (dp d) -> p dp d", d=3)
    for k in range(4):
        base = 51 * k  # block k: coord rows at 51k + 12d' + d, ones at +36+d
        nc.vector.tensor_copy(
            stage[:, base : base + 36].rearrange(
                "p (dp c) -> p dp c", c=12)[:, :, 0:3], rtv)
        nc.vector.tensor_copy(stage[:, base + 36 : base + 39], nreye[:])
    rt16 = cpool.tile([16, 12], F32)
    nc.sync.dma_start(rt16[:], stage[:])

    # ---- projection: vca[p, (ft, k, d)] = [w;1]^T @ rt4 per corner ----
    vca = ppool.tile([128, 120], F32)
    with tc.tile_pool(name="pvc", bufs=1, space="PSUM") as psvc:
        vcp = psvc.tile([128, 120], F32)
        for ft in range(NTILE):
            nc.tensor.matmul(
                vcp[:, 12 * ft : 12 * (ft + 1)],
                vgt[:, 128 * ft : 128 * (ft + 1)],
                rt16[:],
                start=True,
                stop=True,
            )
        nc.vector.tensor_copy(vca[:], vcp[:])

    # junk write that depends on vca: pins the xgrid DMA behind the
    # projection in the scheduler so its long transfer cannot delay rt16's
    nc.vector.tensor_copy(xx[:, 0:1], vca[:, 0:1])
    nc.sync.dma_start(xx[:], xg_d.ap())
    vcav = vca[:].rearrange("p (c d) -> p c d", d=3)
    vx, vy, vz = vcav[:, :, 0], vcav[:, :, 1], vcav[:, :, 2]

    # perspective divide (raw reciprocal; interval margins tolerate ~3e-3)
    dn = ppool.tile([128, 40], F32)
    nc.vector.tensor_scalar(dn[:], vz, TAN_T, EPS, OP.mult, OP.add)
    rc = ppool.tile([128, 40], F32)
    nc.vector.reciprocal(rc[:], dn[:])
    xn = ppool.tile([128, 40], F32)
    nc.vector.tensor_tensor(xn[:], vx, rc[:], OP.mult)
    yn = ppool.tile([128, 40], F32)
    nc.vector.tensor_tensor(yn[:], vy, rc[:], OP.mult)

    # visibility: all corner z > 0 (on Pool)
    vz4 = vca[:].rearrange("p (ft k d) -> p ft k d", k=4, d=3)
    mz1 = ppool.tile([128, 10], F32)
    nc.vector.tensor_tensor(mz1[:], vz4[:, :, 0, 2], vz4[:, :, 1, 2], OP.min)
    mz = ppool.tile([128, 10], F32)
    nc.vector.tensor_tensor(mz[:], mz1[:], vz4[:, :, 2, 2], OP.min)
    vg = ppool.tile([128, 10], F32)
    nc.vector.tensor_scalar(vg[:], mz[:], 0.0, None, OP.is_gt)

    # ---- edge coefficients [128, 30] in (ft, k) layout ----
    xn4 = xn[:].rearrange("p (ft k) -> p ft k", k=4)
    yn4 = yn[:].rearrange("p (ft k) -> p ft k", k=4)
    xk, xk1 = xn4[:, :, 0:3], xn4[:, :, 1:4]
    yk, yk1 = yn4[:, :, 0:3], yn4[:, :, 1:4]

    def t30(name, eng=None):
        return ppool.tile([128, 30], F32, name=name, tag=name)

    A = t30("A")
    Av = A[:].rearrange("p (ft k) -> p ft k", k=3)
    nc.vector.tensor_tensor(Av, yk, yk1, OP.subtract)
    Bc = t30("Bc")
    Bv = Bc[:].rearrange("p (ft k) -> p ft k", k=3)
    nc.vector.tensor_tensor(Bv, xk1, xk, OP.subtract)
    p1 = t30("p1")
    nc.gpsimd.tensor_tensor(p1[:].rearrange("p (ft k) -> p ft k", k=3), xk,
                            yk1, OP.mult)
    p2 = t30("p2")
    nc.gpsimd.tensor_tensor(p2[:].rearrange("p (ft k) -> p ft k", k=3), yk,
                            xk1, OP.mult)
    C = t30("C")
    nc.gpsimd.tensor_tensor(C[:], p1[:], p2[:], OP.subtract)

    Cv = C[:].rearrange("p (ft k) -> p ft k", k=3)
    S1 = ppool.tile([128, 10], F32, name="S1")
    nc.gpsimd.tensor_tensor(S1[:], Cv[:, :, 0], Cv[:, :, 1], OP.add)
    S = ppool.tile([128, 10], F32, name="S")
    nc.gpsimd.tensor_tensor(S[:], S1[:], Cv[:, :, 2], OP.add)

    # masks (Pool side-chain)
    w = t30("w")
    nc.gpsimd.tensor_tensor(w[:].rearrange("p (ft k) -> p ft k", k=3), Av,
                            S[:].unsqueeze(2).broadcast_to([128, 10, 3]),
                            OP.mult)
    mpos = t30("mpos")
    nc.vector.tensor_scalar(mpos[:], w[:], 0.0, None, OP.is_gt)
    mneg = t30("mneg")
    nc.vector.tensor_scalar(mneg[:], w[:], 0.0, None, OP.is_lt)
    offlo = t30("offlo")
    nc.vector.tensor_scalar(offlo[:], mpos[:], BIG, -BIG, OP.mult, OP.add)
    offnh = t30("offnh")
    nc.vector.tensor_scalar(offnh[:], mneg[:], BIG, -BIG, OP.mult, OP.add)
    mnegN = t30("mnegN")
    nc.vector.tensor_scalar(mnegN[:], mneg[:], -1.0, None, OP.mult)

    sne = ppool.tile([128, 10], F32, name="sne")
    nc.vector.tensor_scalar(sne[:], S[:], 0.0, None, OP.not_equal)
    visq = ppool.tile([128, 10], F32, name="visq")
    nc.gpsimd.tensor_tensor(visq[:], vg[:], sne[:], OP.mult)
    ivq = ppool.tile([128, 10], F32, name="ivq")
    nc.vector.tensor_scalar(ivq[:], visq[:], -2.0 * BIG, 2.0 * BIG, OP.mult,
                            OP.add)
    ivqN = ppool.tile([128, 10], F32, name="ivqN")
    nc.vector.tensor_scalar(ivqN[:], visq[:], 2.0 * BIG, -2.0 * BIG, OP.mult,
                            OP.add)

    # reciprocal side (DVE)
    iseq = t30("iseq")
    nc.vector.tensor_scalar(iseq[:], A[:], 0.0, None, OP.is_equal)
    Asafe = t30("Asafe")
    nc.vector.tensor_tensor(Asafe[:], A[:], iseq[:], OP.add)
    r0 = t30("r0")
    nc.vector.reciprocal(r0[:], Asafe[:])
    nr = t30("nr")
    nc.vector.tensor_scalar(nr[:], r0[:], -1.0, None, OP.mult)
    u = t30("u")
    nc.vector.tensor_tensor(u[:], Bc[:], nr[:], OP.mult)
    v = t30("v")
    nc.vector.tensor_tensor(v[:], C[:], nr[:], OP.mult)

    # (u, v) -> interleaved lhsT staging tiles [128, 60]: col 2m = u_m, 2m+1 = v_m
    uvlo = ppool.tile([128, 60], F32, name="uvlo")
    uvlov = uvlo[:].rearrange("p (m two) -> p m two", two=2)
    uvnh = ppool.tile([128, 60], F32, name="uvnh")
    uvnhv = uvnh[:].rearrange("p (m two) -> p m two", two=2)

    # lower side: ulo = u*mpos ; vlo = v*mpos - BIG*(1-mpos) + ivq
    nc.vector.tensor_tensor(uvlov[:, :, 0], u[:], mpos[:], OP.mult)
    vlo1 = t30("vlo1")
    nc.vector.tensor_tensor(vlo1[:], v[:], mpos[:], OP.mult)
    vlo2 = t30("vlo2")
    nc.vector.tensor_tensor(vlo2[:], vlo1[:], offlo[:], OP.add)
    nc.vector.tensor_tensor(
        uvlov[:, :, 1].rearrange("p (ft k) -> p ft k", k=3),
        vlo2[:].rearrange("p (ft k) -> p ft k", k=3),
        ivq[:].unsqueeze(2).broadcast_to([128, 10, 3]), OP.add)

    # negated upper side: unh = -u*mneg ; vnh = -v*mneg - BIG*(1-mneg) - ivq
    nc.vector.tensor_tensor(uvnhv[:, :, 0], u[:], mnegN[:], OP.mult)
    vnh1 = t30("vnh1")
    nc.vector.tensor_tensor(vnh1[:], v[:], mnegN[:], OP.mult)
    vnh2 = t30("vnh2")
    nc.vector.tensor_tensor(vnh2[:], vnh1[:], offnh[:], OP.add)
    nc.vector.tensor_tensor(
        uvnhv[:, :, 1].rearrange("p (ft k) -> p ft k", k=3),
        vnh2[:].rearrange("p (ft k) -> p ft k", k=3),
        ivqN[:].unsqueeze(2).broadcast_to([128, 10, 3]), OP.add)

    # ---- T planes via PE: transpose (u,v), matmul against constant basis ----
    TLOs = gpool.tile([128, 1920], BF16)
    TNHs = gpool.tile([128, 1920], BF16)
    with tc.tile_pool(name="ptr", bufs=2, space="PSUM") as ptr:
        uvloT = ptr.tile([60, 128], F32, tag="uvT")
        nc.tensor.transpose(uvloT[:], uvlo[:], idm[:])
        uvloB = gpool.tile([60, 128], BF16)
        nc.scalar.activation(uvloB[:], uvloT[:], AF.Copy)
        uvnhT = ptr.tile([60, 128], F32, tag="uvT")
        nc.tensor.transpose(uvnhT[:], uvnh[:], idm[:])
        uvnhB = gpool.tile([60, 128], BF16)
        nc.scalar.activation(uvnhB[:], uvnhT[:], AF.Copy)
    with tc.tile_pool(name="ptp", bufs=2, space="PSUM") as ptp:
        TLOp = ptp.tile([128, 1920], F32, tag="tp")
        for q in range(4):
            nc.tensor.matmul(TLOp[:, 480 * q : 480 * (q + 1)], uvloB[:],
                             tb[:, 480 * q : 480 * (q + 1)], start=True,
                             stop=True)
        nc.scalar.activation(TLOs[:], TLOp[:], AF.Copy)
        TNHp = ptp.tile([128, 1920], F32, tag="tp")
        for q in range(4):
            nc.tensor.matmul(TNHp[:, 480 * q : 480 * (q + 1)], uvnhB[:],
                             tb[:, 480 * q : 480 * (q + 1)], start=True,
                             stop=True)
        nc.vector.tensor_copy(TNHs[:], TNHp[:])

    # ---- chains -> LH [128, 1280]: cols (s, ft, i); s=0: lo, s=1: -hi ----
    TLOv = TLOs[:].rearrange("p (ft k i) -> p ft k i", k=3, i=IMG)
    TNHv = TNHs[:].rearrange("p (ft k i) -> p ft k i", k=3, i=IMG)
    LH = gpool.tile([128, 2 * 640], BF16)
    lo1 = gpool.tile([128, 640], BF16)
    nc.vector.tensor_tensor(lo1[:], TLOv[:, :, 0, :], TLOv[:, :, 1, :], OP.max)
    nc.vector.tensor_tensor(
        LH[:, 0:640].rearrange("p (ft i) -> p ft i", i=IMG),
        lo1[:].rearrange("p (ft i) -> p ft i", i=IMG), TLOv[:, :, 2, :],
        OP.max)
    nh1 = gpool.tile([128, 640], BF16)
    nc.vector.tensor_tensor(nh1[:], TNHv[:, :, 0, :], TNHv[:, :, 1, :], OP.max)
    nh2 = gpool.tile([128, 640], BF16)
    nc.vector.tensor_tensor(
        nh2[:].rearrange("p (ft i) -> p ft i", i=IMG),
        nh1[:].rearrange("p (ft i) -> p ft i", i=IMG), TNHv[:, :, 2, :],
        OP.max)
    # canonicalize empty rows: -hi' = min(-hi, -lo)
    nlo = gpool.tile([128, 640], BF16)
    nc.vector.tensor_scalar(nlo[:], LH[:, 0:640], -1.0, None, OP.mult)
    nc.vector.tensor_tensor(LH[:, 640:1280], nh2[:], nlo[:], OP.min)

    # ---- raster: per face-tile one combined is_ge + 16 accum matmuls ----
    # Junk "warmer" matmuls keep the PE p-state ramped: a pre-raster burst
    # while the first compare runs, plus a couple per face-tile to bridge the
    # compare/accumulate rate gap without the engine ever going idle.
    LHv = LH[:].rearrange("p (s ft i) -> p s ft i", s=2, ft=NTILE)
    spool = ctx.enter_context(tc.tile_pool(name="ghp", bufs=3))
    pscnt = ctx.enter_context(tc.tile_pool(name="pcnt", bufs=1, space="PSUM"))
    pwarm = ctx.enter_context(tc.tile_pool(name="pwarm", bufs=1, space="PSUM"))
    cnt8 = pscnt.tile([8, 512], F32, tag="cnt8")
    wps = pwarm.tile([128, 480], F32, tag="wps")

    def warm(n):
        for wq in range(n):
            nc.tensor.matmul(wps[:], uvloB[:], tb[:, 0:480], start=True,
                             stop=True)

    warm(10)
    nmm = 0
    NMM = NTILE * 16
    for ft in range(NTILE):
        ghp = spool.tile([128, 2 * NPIX], BF16, tag="ghp")
        lhb = LHv[:, :, ft, :].unsqueeze(2).broadcast_to([128, 2, IMG, IMG])
        nc.vector.tensor_tensor(
            ghp[:].rearrange("p (s j i) -> p s j i", s=2, j=IMG), xxv, lhb,
            OP.is_ge)
        for c in range(16):
            q = c % 8
            nc.tensor.matmul(cnt8[:, :], oh[:, 8 - q : 16 - q],
                             ghp[:, 512 * c : 512 * (c + 1)],
                             start=(nmm == 0), stop=(nmm == NMM - 1))
            nmm += 1
        if ft < NTILE - 1:
            warm(2)

    # ---- threshold: covered iff cnt >= NF + 1 ----
    silb = gpool.tile([8, 512], F32)
    nc.vector.tensor_scalar(silb[:], cnt8[:], NF + 0.5, None, OP.is_gt)
    nc.sync.dma_start(sil_d.ap(), silb[:])


_NC = None


def _get_program():
    global _NC
    if _NC is None:
        nc = bacc.Bacc(
            "TRN2",
            target_bir_lowering=False,
            debug=False,
            enable_asserts=False,
            num_devices=B,
        )
        from contextlib import ExitStack

        with tile.TileContext(nc) as tc:
            with ExitStack() as ctx:
                build_kernel(ctx, tc)
        nc.compile()
        _NC = nc
    return _NC


def _consts():
    """Input-independent constant tables (pixel grid, t-plane basis)."""
    j = np.arange(IMG, dtype=np.float32)
    xs = (2.0 * j - 63.0) / 64.0                      # exact in bf16
    ys = (63.0 - 2.0 * j) / 64.0
    xg = np.empty((2, IMG, IMG), dtype=np.float32)
    xg[0] = xs[:, None]
    xg[1] = -xs[:, None]
    xgrid = np.broadcast_to(xg.reshape(1, 2 * NPIX), (128, 2 * NPIX))
    xgrid = np.ascontiguousarray(xgrid).astype(ml_dtypes.bfloat16)
    tb = np.zeros((60, 1920), dtype=np.float32)
    for m in range(30):
        tb[2 * m, m * 64 : (m + 1) * 64] = ys
        tb[2 * m + 1, m * 64 : (m + 1) * 64] = 1.0
    tbasis = tb.astype(ml_dtypes.bfloat16)
    return xgrid, tbasis


def _host_layout(vertices, faces):
    """Pure indexing: gather per-face-corner vertices into [16, 1280] where
    row 4k+d / column ft*128+p holds coord d (d=3: 1.0) of corner k of face
    ft*128+p; corners are (a, b, c, a)."""
    faces4 = np.concatenate([faces, faces[:, :1]], axis=1)  # [1280, 4]
    out = []
    for b in range(B):
        vg = vertices[b][faces4]                      # [1280, 4, 3]
        vg4 = np.concatenate(
            [vg, np.ones((NF, 4, 1), dtype=np.float32)], axis=2)  # [1280,4,4]
        out.append(np.ascontiguousarray(
            vg4.transpose(1, 2, 0).reshape(16, NF).astype(np.float32)))
    return out


def kernel(vertices, viewpoints, faces, img_size):
    vertices = np.asarray(vertices, dtype=np.float32)
    viewpoints = np.asarray(viewpoints, dtype=np.float32)
    faces = np.asarray(faces, dtype=np.int32)
    assert int(img_size) == IMG and vertices.shape == (B, V, 3)

    nc = _get_program()
    vgts = _host_layout(vertices, faces)
    xgrid, tbasis = _consts()
    in_maps = [
        {"vgt16": vgts[b], "eye": np.ascontiguousarray(viewpoints[b]),
         "xgrid": xgrid, "tbasis": tbasis}
        for b in range(B)
    ]
    res = run_bass_kernel_spmd(nc, in_maps, core_ids=list(range(B)))
    # device pixel order is (j, i): transpose back to raster (i, j)
    sil = np.stack([
        res.results[b]["sil"].reshape(IMG, IMG).T for b in range(B)
    ])
    return sil.reshape(B, 1, IMG, IMG).astype(np.float32)


if __name__ == "__main__":
    rng = np.random.default_rng(0)
    verts = rng.standard_normal((B, V, 3), dtype=np.float32) * 0.5
    vps = rng.standard_normal((B, 3), dtype=np.float32)
    fcs = rng.integers(0, V, (NF, 3), dtype=np.int32)
    out = kernel(verts, vps, fcs, IMG)
    print(out.shape, out.sum())


# revision 33
# speedup vs baseline: 2.7627x; 1.0466x over previous
"""Trainium2 Bass kernel for nn_Mesh_Renderer: silhouette via scanline intervals.

Data-parallel over batch (core b renders view b). Host work is layout only
(gather vertices[faces], constant grid/basis tables, transpose the returned
image). All input-dependent math on device.

Device algorithm (per core):
  1. look_at camera basis from eye; projection folded as [w;1]^T @ [R^T; -R@eye]
     (40 K=4 f32 matmuls), perspective divide -> per-corner (xn, yn) [128, 40].
  2. Edge coefficients per (face, edge): e = A x + B y + C. For each pixel row
     y_i the face coverage in x is an interval [lo, hi]:
       t_k(i) = -(B_k y_i + C_k)/A_k ; edge k bounds from below iff
       sign(2*area)*A_k > 0. lo = max over lower-edges, -hi = max over upper
       (negated). Invisible/degenerate faces forced to a contributes-nothing
       interval via +-BIG offsets folded into the per-edge (u, v) small tiles;
       empty rows canonicalized with -hi' = min(-hi, -lo) (point interval).
     The t-planes t = u*y + v are evaluated by PE against a constant
     block-diagonal basis (tbasis), with (u, v) PE-transposed into lhsT.
  3. Raster: count(i,j) = sum_f([x_j >= lo] + [x_j <= hi]) = F + #covering.
     One DVE is_ge over [128, 8192] per 128-face tile computes both compares
     ([x | -x] vs [lo | -hi] broadcast over j); PE ones-matmuls accumulate
     over faces into PSUM cnt8 [8, 512] (sliding-onehot lhsT selects the row).
  4. silhouette = cnt >= F+1; DMA out; host transposes (j,i)->(i,j).
"""

import sys

if "/opt/trn_rl_repo" not in sys.path:
    sys.path.insert(0, "/opt/trn_rl_repo")

import ml_dtypes
import numpy as np

import concourse.bacc as bacc
import concourse.tile as tile
from concourse import mybir
from concourse.bass_utils import run_bass_kernel_spmd

F32 = mybir.dt.float32
BF16 = mybir.dt.bfloat16
I32 = mybir.dt.int32
OP = mybir.AluOpType
AF = mybir.ActivationFunctionType

B, V, NF, IMG = 8, 642, 1280, 64
NPIX = IMG * IMG          # 4096
NTILE = NF // 128         # 10 face tiles
NCOL = NF * 4             # 5120 gathered corners (a, b, c, a)
EPS = 1e-8
BIG = 1.0e30
TAN_T = float(np.tan(np.deg2rad(np.float32(15.0)).astype(np.float32)))


def _normalize3(nc, pool, v, name):
    """v [1,3] f32 -> v * rsqrt(sum v^2); margins cover the eps difference."""
    sq = pool.tile([1, 3], F32, name=f"{name}_sq")
    nc.vector.tensor_tensor(sq[:], v[:], v[:], OP.mult)
    s = pool.tile([1, 1], F32, name=f"{name}_s")
    nc.vector.tensor_reduce(s[:], sq[:], mybir.AxisListType.X, OP.add)
    n = pool.tile([1, 1], F32, name=f"{name}_n")
    nc.scalar.activation(n[:], s[:], AF.Sqrt)
    r = pool.tile([1, 1], F32, name=f"{name}_r")
    nc.vector.reciprocal(r[:], n[:])
    out = pool.tile([1, 3], F32, name=f"{name}_out")
    nc.vector.tensor_scalar(out[:], v[:], r[:], None, OP.mult)
    return out


def _cross3(nc, pool, a, b, name):
    a2 = pool.tile([1, 6], F32, name=f"{name}_a2")
    nc.vector.tensor_copy(a2[:, 0:3], a[:])
    nc.vector.tensor_copy(a2[:, 3:6], a[:])
    b2 = pool.tile([1, 6], F32, name=f"{name}_b2")
    nc.vector.tensor_copy(b2[:, 0:3], b[:])
    nc.vector.tensor_copy(b2[:, 3:6], b[:])
    m1 = pool.tile([1, 3], F32, name=f"{name}_m1")
    nc.vector.tensor_tensor(m1[:], a2[:, 1:4], b2[:, 2:5], OP.mult)
    m2 = pool.tile([1, 3], F32, name=f"{name}_m2")
    nc.vector.tensor_tensor(m2[:], a2[:, 2:5], b2[:, 1:4], OP.mult)
    out = pool.tile([1, 3], F32, name=f"{name}_out")
    nc.vector.tensor_tensor(out[:], m1[:], m2[:], OP.subtract)
    return out


def build_kernel(ctx, tc):
    nc = tc.nc
    vgt_d = nc.dram_tensor("vgt16", [16, NF], F32, kind="ExternalInput")
    eye_d = nc.dram_tensor("eye", [3], F32, kind="ExternalInput")
    xg_d = nc.dram_tensor("xgrid", [128, 2 * NPIX], BF16, kind="ExternalInput")
    tb_d = nc.dram_tensor("tbasis", [60, 1920], BF16, kind="ExternalInput")
    xb_d = nc.dram_tensor("xb65", [65, NPIX], BF16, kind="ExternalInput")
    sil_d = nc.dram_tensor("sil", [NPIX], F32, kind="ExternalOutput")

    cpool = ctx.enter_context(tc.tile_pool(name="cam", bufs=1))
    ppool = ctx.enter_context(tc.tile_pool(name="proj", bufs=1))
    gpool = ctx.enter_context(tc.tile_pool(name="grid", bufs=1))

    # ---- input DMAs ----
    eyeR = cpool.tile([1, 3], F32)
    nc.sync.dma_start(eyeR[:], eye_d.ap())
    vgt = gpool.tile([16, NF], F32)
    nc.sync.dma_start(vgt[:], vgt_d.ap())
    tb = gpool.tile([60, 1920], BF16)
    nc.sync.dma_start(tb[:], tb_d.ap())
    xb65 = gpool.tile([65, NPIX], BF16)
    nc.sync.dma_start(xb65[:], xb_d.ap())
    xx = gpool.tile([128, 2 * NPIX], BF16)
    xxv = xx[:].rearrange("p (s j i) -> p s j i", s=2, j=IMG)

    # identity for PE transposes (iotas on Pool, rest tiny)
    iop = gpool.tile([128, 1], I32)
    nc.gpsimd.iota(iop[:], pattern=[[1, 1]], base=0, channel_multiplier=1)
    iopf = gpool.tile([128, 1], F32)
    nc.vector.tensor_copy(iopf[:], iop[:])
    iof = gpool.tile([128, 128], I32)
    nc.gpsimd.iota(iof[:], pattern=[[1, 128]], base=0, channel_multiplier=0)
    ioff = gpool.tile([128, 128], F32)
    nc.vector.tensor_copy(ioff[:], iof[:])
    idm = gpool.tile([128, 128], F32)
    nc.vector.tensor_scalar(idm[:], ioff[:], iopf[:], None, OP.is_equal)

    # sliding one-hot for row-targeted PE accumulation
    oh = gpool.tile([128, 16], BF16)
    nc.gpsimd.memset(oh[:], 0.0)
    nc.gpsimd.memset(oh[:, 8:9], 2.0)
    oh1 = gpool.tile([128, 16], BF16)
    nc.gpsimd.memset(oh1[:], 0.0)
    nc.gpsimd.memset(oh1[:, 8:9], 1.0)

    # ---- camera basis (partition 0, tiny tiles) ----
    # x_ax dir = cross(up, z) = cross(up, -eye) up to positive scale, so the
    # x/y chain runs off -eye directly; z-normalize is off the critical path.
    nege = cpool.tile([1, 3], F32)
    nc.vector.tensor_scalar(nege[:], eyeR[:], -1.0, None, OP.mult)
    xr = cpool.tile([1, 3], F32)
    nc.vector.memset(xr[:], 0.0)
    nc.vector.tensor_copy(xr[:, 0:1], nege[:, 2:3])
    nc.vector.tensor_scalar(xr[:, 2:3], nege[:, 0:1], -1.0, None, OP.mult)
    x_ax = _normalize3(nc, cpool, xr, "nx")
    z_ax = _normalize3(nc, cpool, nege, "nz")
    y_ax = _cross3(nc, cpool, z_ax, x_ax, "cy")

    # rt16 = 4 diagonal copies of rt4 = [R^T; -(eye^T @ R^T)] (one per corner),
    # staged row-major on partition 0 and reshaped by a single DMA.
    # stage[0, r*12 + c]; block k: rows 4k+d' cols 3k+d hold R[d, d'] and row
    # 4k+3 holds -Reye[d].
    rtT9 = cpool.tile([1, 9], F32)   # rtT9[0, 3*d' + d] = axis_d[d']
    for d, axis in enumerate([x_ax, y_ax, z_ax]):
        nc.vector.tensor_copy(
            rtT9[:].rearrange("p (dp d) -> p dp d", d=3)[:, :, d], axis[:])
    # -Reye[d] = -sum_dp eye[dp] * R^T[dp, d] via elementwise + X-reduce
    el = cpool.tile([1, 9], F32)   # (d, dp) layout
    nc.vector.tensor_tensor(
        el[:].rearrange("p (d dp) -> p d dp", dp=3),
        rtT9[:].rearrange("p (dp d) -> p d dp", d=3),
        eyeR[:].unsqueeze(1).broadcast_to([1, 3, 3]), OP.mult)
    nreye0 = cpool.tile([1, 3], F32)
    nc.vector.tensor_reduce(nreye0[:], el[:].rearrange(
        "p (d dp) -> p d dp", dp=3), mybir.AxisListType.X, OP.add)
    nreye = cpool.tile([1, 3], F32)
    nc.vector.tensor_scalar(nreye[:], nreye0[:], -1.0, None, OP.mult)
    stage = cpool.tile([1, 192], F32)
    nc.vector.memset(stage[:], 0.0)
    rtv = rtT9[:].rearrange("p (dp d) -> p dp d", d=3)
    for k in range(4):
        base = 51 * k  # block k: coord rows at 51k + 12d' + d, ones at +36+d
        nc.vector.tensor_copy(
            stage[:, base : base + 36].rearrange(
                "p (dp c) -> p dp c", c=12)[:, :, 0:3], rtv)
        nc.vector.tensor_copy(stage[:, base + 36 : base + 39], nreye[:])
    rt16 = cpool.tile([16, 12], F32)
    nc.sync.dma_start(rt16[:], stage[:])

    # ---- projection: vca[p, (ft, k, d)] = [w;1]^T @ rt4 per corner ----
    vca = ppool.tile([128, 120], F32)
    with tc.tile_pool(name="pvc", bufs=1, space="PSUM") as psvc:
        vcp = psvc.tile([128, 120], F32)
        for ft in range(NTILE):
            nc.tensor.matmul(
                vcp[:, 12 * ft : 12 * (ft + 1)],
                vgt[:, 128 * ft : 128 * (ft + 1)],
                rt16[:],
                start=True,
                stop=True,
            )
        nc.vector.tensor_copy(vca[:], vcp[:])

    # junk write that depends on vca: pins the xgrid DMA behind the
    # projection in the scheduler so its long transfer cannot delay rt16's
    nc.vector.tensor_copy(xx[:, 0:1], vca[:, 0:1])
    nc.sync.dma_start(xx[:], xg_d.ap())
    # keep PE p-state ramped between projection and the T matmuls
    with tc.tile_pool(name="pwarm0", bufs=1, space="PSUM") as pwarm0:
        wps0 = pwarm0.tile([128, 480], F32, tag="wps0")
        for _ in range(14):
            nc.tensor.matmul(wps0[:], tb[:, 0:128], tb[:, 0:480], start=True,
                             stop=True)
    vcav = vca[:].rearrange("p (c d) -> p c d", d=3)
    vx, vy, vz = vcav[:, :, 0], vcav[:, :, 1], vcav[:, :, 2]

    # perspective divide (raw reciprocal; interval margins tolerate ~3e-3)
    dn = ppool.tile([128, 40], F32)
    nc.vector.tensor_scalar(dn[:], vz, TAN_T, EPS, OP.mult, OP.add)
    rc = ppool.tile([128, 40], F32)
    nc.vector.reciprocal(rc[:], dn[:])
    xn = ppool.tile([128, 40], F32)
    nc.vector.tensor_tensor(xn[:], vx, rc[:], OP.mult)
    yn = ppool.tile([128, 40], F32)
    nc.vector.tensor_tensor(yn[:], vy, rc[:], OP.mult)

    # visibility: all corner z > 0 (on Pool)
    vz4 = vca[:].rearrange("p (ft k d) -> p ft k d", k=4, d=3)
    mz1 = ppool.tile([128, 10], F32)
    nc.vector.tensor_tensor(mz1[:], vz4[:, :, 0, 2], vz4[:, :, 1, 2], OP.min)
    mz = ppool.tile([128, 10], F32)
    nc.vector.tensor_tensor(mz[:], mz1[:], vz4[:, :, 2, 2], OP.min)
    vg = ppool.tile([128, 10], F32)
    nc.vector.tensor_scalar(vg[:], mz[:], 0.0, None, OP.is_gt)

    # ---- edge coefficients [128, 30] in (ft, k) layout ----
    xn4 = xn[:].rearrange("p (ft k) -> p ft k", k=4)
    yn4 = yn[:].rearrange("p (ft k) -> p ft k", k=4)
    xk, xk1 = xn4[:, :, 0:3], xn4[:, :, 1:4]
    yk, yk1 = yn4[:, :, 0:3], yn4[:, :, 1:4]

    def t30(name, eng=None):
        return ppool.tile([128, 30], F32, name=name, tag=name)

    A = t30("A")
    Av = A[:].rearrange("p (ft k) -> p ft k", k=3)
    nc.vector.tensor_tensor(Av, yk, yk1, OP.subtract)
    Bc = t30("Bc")
    Bv = Bc[:].rearrange("p (ft k) -> p ft k", k=3)
    nc.vector.tensor_tensor(Bv, xk1, xk, OP.subtract)
    p1 = t30("p1")
    nc.gpsimd.tensor_tensor(p1[:].rearrange("p (ft k) -> p ft k", k=3), xk,
                            yk1, OP.mult)
    p2 = t30("p2")
    nc.gpsimd.tensor_tensor(p2[:].rearrange("p (ft k) -> p ft k", k=3), yk,
                            xk1, OP.mult)
    C = t30("C")
    nc.gpsimd.tensor_tensor(C[:], p1[:], p2[:], OP.subtract)

    Cv = C[:].rearrange("p (ft k) -> p ft k", k=3)
    S1 = ppool.tile([128, 10], F32, name="S1")
    nc.gpsimd.tensor_tensor(S1[:], Cv[:, :, 0], Cv[:, :, 1], OP.add)
    S = ppool.tile([128, 10], F32, name="S")
    nc.gpsimd.tensor_tensor(S[:], S1[:], Cv[:, :, 2], OP.add)

    # masks (Pool side-chain)
    w = t30("w")
    nc.gpsimd.tensor_tensor(w[:].rearrange("p (ft k) -> p ft k", k=3), Av,
                            S[:].unsqueeze(2).broadcast_to([128, 10, 3]),
                            OP.mult)
    mpos = t30("mpos")
    nc.vector.tensor_scalar(mpos[:], w[:], 0.0, None, OP.is_gt)
    mneg = t30("mneg")
    nc.vector.tensor_scalar(mneg[:], w[:], 0.0, None, OP.is_lt)
    offlo = t30("offlo")
    nc.vector.tensor_scalar(offlo[:], mpos[:], BIG, -BIG, OP.mult, OP.add)
    offnh = t30("offnh")
    nc.vector.tensor_scalar(offnh[:], mneg[:], BIG, -BIG, OP.mult, OP.add)
    mnegN = t30("mnegN")
    nc.vector.tensor_scalar(mnegN[:], mneg[:], -1.0, None, OP.mult)

    sne = ppool.tile([128, 10], F32, name="sne")
    nc.vector.tensor_scalar(sne[:], S[:], 0.0, None, OP.not_equal)
    visq = ppool.tile([128, 10], F32, name="visq")
    nc.gpsimd.tensor_tensor(visq[:], vg[:], sne[:], OP.mult)
    ivq = ppool.tile([128, 10], F32, name="ivq")
    nc.vector.tensor_scalar(ivq[:], visq[:], -2.0 * BIG, 2.0 * BIG, OP.mult,
                            OP.add)
    ivqN = ppool.tile([128, 10], F32, name="ivqN")
    nc.vector.tensor_scalar(ivqN[:], visq[:], 2.0 * BIG, -2.0 * BIG, OP.mult,
                            OP.add)

    # reciprocal side (DVE)
    iseq = t30("iseq")
    nc.vector.tensor_scalar(iseq[:], A[:], 0.0, None, OP.is_equal)
    Asafe = t30("Asafe")
    nc.vector.tensor_tensor(Asafe[:], A[:], iseq[:], OP.add)
    r0 = t30("r0")
    nc.vector.reciprocal(r0[:], Asafe[:])
    nr = t30("nr")
    nc.vector.tensor_scalar(nr[:], r0[:], -1.0, None, OP.mult)
    u = t30("u")
    nc.vector.tensor_tensor(u[:], Bc[:], nr[:], OP.mult)
    v = t30("v")
    nc.vector.tensor_tensor(v[:], C[:], nr[:], OP.mult)

    # (u, v) -> interleaved lhsT staging tiles [128, 60]: col 2m = u_m, 2m+1 = v_m
    uvlo = ppool.tile([128, 60], F32, name="uvlo")
    uvlov = uvlo[:].rearrange("p (m two) -> p m two", two=2)
    uvnh = ppool.tile([128, 60], F32, name="uvnh")
    uvnhv = uvnh[:].rearrange("p (m two) -> p m two", two=2)

    # lower side: ulo = u*mpos ; vlo = v*mpos - BIG*(1-mpos) + ivq
    nc.vector.tensor_tensor(uvlov[:, :, 0], u[:], mpos[:], OP.mult)
    vlo1 = t30("vlo1")
    nc.vector.tensor_tensor(vlo1[:], v[:], mpos[:], OP.mult)
    vlo2 = t30("vlo2")
    nc.vector.tensor_tensor(vlo2[:], vlo1[:], offlo[:], OP.add)
    nc.vector.tensor_tensor(
        uvlov[:, :, 1].rearrange("p (ft k) -> p ft k", k=3),
        vlo2[:].rearrange("p (ft k) -> p ft k", k=3),
        ivq[:].unsqueeze(2).broadcast_to([128, 10, 3]), OP.add)

    # negated upper side: unh = -u*mneg ; vnh = -v*mneg - BIG*(1-mneg) - ivq
    nc.vector.tensor_tensor(uvnhv[:, :, 0], u[:], mnegN[:], OP.mult)
    vnh1 = t30("vnh1")
    nc.vector.tensor_tensor(vnh1[:], v[:], mnegN[:], OP.mult)
    vnh2 = t30("vnh2")
    nc.vector.tensor_tensor(vnh2[:], vnh1[:], offnh[:], OP.add)
    nc.vector.tensor_tensor(
        uvnhv[:, :, 1].rearrange("p (ft k) -> p ft k", k=3),
        vnh2[:].rearrange("p (ft k) -> p ft k", k=3),
        ivqN[:].unsqueeze(2).broadcast_to([128, 10, 3]), OP.add)

    # ---- T planes via PE: transpose (u,v), matmul against constant basis ----
    TLOs = gpool.tile([128, 1920], BF16)
    TNHs = gpool.tile([128, 1920], BF16)
    with tc.tile_pool(name="ptr", bufs=2, space="PSUM") as ptr:
        uvloT = ptr.tile([60, 128], F32, tag="uvT")
        nc.tensor.transpose(uvloT[:], uvlo[:], idm[:])
        uvloB = gpool.tile([60, 128], BF16)
        nc.scalar.activation(uvloB[:], uvloT[:], AF.Copy)
        uvnhT = ptr.tile([60, 128], F32, tag="uvT")
        nc.tensor.transpose(uvnhT[:], uvnh[:], idm[:])
        uvnhB = gpool.tile([60, 128], BF16)
        nc.scalar.activation(uvnhB[:], uvnhT[:], AF.Copy)
    with tc.tile_pool(name="ptp", bufs=2, space="PSUM") as ptp:
        TLOp = ptp.tile([128, 1920], F32, tag="tp")
        for q in range(4):
            nc.tensor.matmul(TLOp[:, 480 * q : 480 * (q + 1)], uvloB[:],
                             tb[:, 480 * q : 480 * (q + 1)], start=True,
                             stop=True)
        nc.scalar.activation(TLOs[:], TLOp[:], AF.Copy)
        TNHp = ptp.tile([128, 1920], F32, tag="tp")
        for q in range(4):
            nc.tensor.matmul(TNHp[:, 480 * q : 480 * (q + 1)], uvnhB[:],
                             tb[:, 480 * q : 480 * (q + 1)], start=True,
                             stop=True)
        nc.vector.tensor_copy(TNHs[:], TNHp[:])

    # ---- chains -> LH [128, 1280]: cols (s, ft, i); s=0: lo, s=1: -hi ----
    TLOv = TLOs[:].rearrange("p (ft k i) -> p ft k i", k=3, i=IMG)
    TNHv = TNHs[:].rearrange("p (ft k i) -> p ft k i", k=3, i=IMG)
    LH = gpool.tile([128, 2 * 640], BF16)
    lo1 = gpool.tile([128, 640], BF16)
    nc.vector.tensor_tensor(lo1[:], TLOv[:, :, 0, :], TLOv[:, :, 1, :], OP.max)
    nc.vector.tensor_tensor(
        LH[:, 0:640].rearrange("p (ft i) -> p ft i", i=IMG),
        lo1[:].rearrange("p (ft i) -> p ft i", i=IMG), TLOv[:, :, 2, :],
        OP.max)
    nh1 = gpool.tile([128, 640], BF16)
    nc.vector.tensor_tensor(nh1[:], TNHv[:, :, 0, :], TNHv[:, :, 1, :], OP.max)
    nh2 = gpool.tile([128, 640], BF16)
    nc.vector.tensor_tensor(
        nh2[:].rearrange("p (ft i) -> p ft i", i=IMG),
        nh1[:].rearrange("p (ft i) -> p ft i", i=IMG), TNHv[:, :, 2, :],
        OP.max)
    # canonicalize empty rows: -hi' = min(-hi, -lo)
    nlo = gpool.tile([128, 640], BF16)
    nc.vector.tensor_scalar(nlo[:], LH[:, 0:640], -1.0, None, OP.mult)
    nc.vector.tensor_tensor(LH[:, 640:1280], nh2[:], nlo[:], OP.min)

    # ---- face-tile 9 goes through PE diff-planes + ACT Sign ----
    # d1 = x - lo, d2 = hi - x as K=65 matmuls vs the constant pixel basis
    # xb65 (rows 0..63 = onehot(i), row 64 = x_j). lhsT rows hold -lo / +hi
    # (PE-transposed from LH) with the x-coefficient in row 64.
    ACT_FT = NTILE - 1
    loP = gpool.tile([128, 65], BF16)
    nc.vector.tensor_copy(loP[:, 0:64], LH[:, 640 - 64 : 640])
    nc.vector.memset(loP[:, 64:65], -1.0)
    hiP = gpool.tile([128, 65], BF16)
    nc.vector.tensor_copy(hiP[:, 0:64], LH[:, 1280 - 64 : 1280])
    nc.vector.memset(hiP[:, 64:65], 1.0)
    idmb = gpool.tile([128, 128], BF16)
    nc.vector.tensor_copy(idmb[:], idm[:])
    lhsT1 = gpool.tile([65, 128], BF16)
    lhsT2 = gpool.tile([65, 128], BF16)
    with tc.tile_pool(name="ptd", bufs=2, space="PSUM") as ptd:
        loT = ptd.tile([65, 128], BF16, tag="dT")
        nc.tensor.transpose(loT[:], loP[:], idmb[:])
        nc.scalar.activation(lhsT1[:], loT[:], AF.Copy, scale=-1.0)
        hiT = ptd.tile([65, 128], BF16, tag="dT")
        nc.tensor.transpose(hiT[:], hiP[:], idmb[:])
        nc.scalar.activation(lhsT2[:], hiT[:], AF.Copy, scale=-1.0)
    sgn = gpool.tile([128, 2 * NPIX], BF16)

    # ---- raster: per face-tile one combined is_ge + 16 accum matmuls ----
    # Junk "warmer" matmuls keep the PE p-state ramped: a pre-raster burst
    # while the first compare runs, plus a couple per face-tile to bridge the
    # compare/accumulate rate gap without the engine ever going idle.
    LHv = LH[:].rearrange("p (s ft i) -> p s ft i", s=2, ft=NTILE)
    spool = ctx.enter_context(tc.tile_pool(name="ghp", bufs=3))
    pscnt = ctx.enter_context(tc.tile_pool(name="pcnt", bufs=1, space="PSUM"))
    pwarm = ctx.enter_context(tc.tile_pool(name="pwarm", bufs=1, space="PSUM"))
    cnt8 = pscnt.tile([8, 512], F32, tag="cnt8")
    wps = pwarm.tile([128, 480], F32, tag="wps")

    def warm(n):
        for wq in range(n):
            nc.tensor.matmul(wps[:], uvloB[:], tb[:, 0:480], start=True,
                             stop=True)

    warm(10)
    pdif = ctx.enter_context(tc.tile_pool(name="pdif", bufs=1, space="PSUM"))
    nmm = 0
    NMM = (NTILE - 1) * 16

    def diff_half(h):
        side, hh = h // 2, h % 2
        lhsT = lhsT1 if side == 0 else lhsT2
        dp = pdif.tile([128, 2048], F32, tag="dp")
        for q in range(4):
            off = 2048 * hh + 512 * q
            nc.tensor.matmul(dp[:, 512 * q : 512 * (q + 1)], lhsT[:],
                             xb65[:, off : off + 512], start=True, stop=True)
        return dp

    def sign_half(h, dp):
        nc.scalar.activation(sgn[:, 2048 * h : 2048 * (h + 1)], dp[:], AF.Sign)

    def sgn_accum(slot):
        for c in range(4 * slot, 4 * slot + 4):
            q = c % 8
            nc.tensor.matmul(cnt8[:, :], oh1[:, 8 - q : 16 - q],
                             sgn[:, 512 * c : 512 * (c + 1)],
                             start=False, stop=False)

    dps = {0: diff_half(0)}
    for ft in range(NTILE - 1):
        ghp = spool.tile([128, 2 * NPIX], BF16, tag="ghp")
        lhb = LHv[:, :, ft, :].unsqueeze(2).broadcast_to([128, 2, IMG, IMG])
        nc.vector.tensor_tensor(
            ghp[:].rearrange("p (s j i) -> p s j i", s=2, j=IMG), xxv, lhb,
            OP.is_ge)
        for c in range(16):
            q = c % 8
            nc.tensor.matmul(cnt8[:, :], oh[:, 8 - q : 16 - q],
                             ghp[:, 512 * c : 512 * (c + 1)],
                             start=(nmm == 0), stop=(nmm == NMM - 1))
            nmm += 1
        if ft <= 3:
            sign_half(ft, dps.pop(ft))
            if ft < 3:
                dps[ft + 1] = diff_half(ft + 1)
        elif ft <= 7:
            sgn_accum(ft - 4)

    # ---- threshold: covered iff cnt >= NF + 1 ----
    silb = gpool.tile([8, 512], F32)
    nc.vector.tensor_scalar(silb[:], cnt8[:], 2.0 * (NF - 128) + 0.5, None,
                            OP.is_gt)
    nc.sync.dma_start(sil_d.ap(), silb[:])


_NC = None


def _get_program():
    global _NC
    if _NC is None:
        nc = bacc.Bacc(
            "TRN2",
            target_bir_lowering=False,
            debug=False,
            enable_asserts=False,
            num_devices=B,
        )
        from contextlib import ExitStack

        with tile.TileContext(nc) as tc:
            with ExitStack() as ctx:
                build_kernel(ctx, tc)
        nc.compile()
        _NC = nc
    return _NC


def _consts():
    """Input-independent constant tables (pixel grid, t-plane basis)."""
    j = np.arange(IMG, dtype=np.float32)
    xs = (2.0 * j - 63.0) / 64.0                      # exact in bf16
    ys = (63.0 - 2.0 * j) / 64.0
    xg = np.empty((2, IMG, IMG), dtype=np.float32)
    xg[0] = xs[:, None]
    xg[1] = -xs[:, None]
    xgrid = np.broadcast_to(xg.reshape(1, 2 * NPIX), (128, 2 * NPIX))
    xgrid = np.ascontiguousarray(xgrid).astype(ml_dtypes.bfloat16)
    tb = np.zeros((60, 1920), dtype=np.float32)
    for m in range(30):
        tb[2 * m, m * 64 : (m + 1) * 64] = ys
        tb[2 * m + 1, m * 64 : (m + 1) * 64] = 1.0
    tbasis = tb.astype(ml_dtypes.bfloat16)
    xb = np.zeros((65, NPIX), dtype=np.float32)
    for i in range(IMG):
        xb[i, i::IMG] = 1.0                    # onehot(i) over (j, i) columns
    xb[64] = np.repeat(xs, IMG)                # x_j
    xb65 = xb.astype(ml_dtypes.bfloat16)
    return xgrid, tbasis, xb65


def _host_layout(vertices, faces):
    """Pure indexing: gather per-face-corner vertices into [16, 1280] where
    row 4k+d / column ft*128+p holds coord d (d=3: 1.0) of corner k of face
    ft*128+p; corners are (a, b, c, a)."""
    faces4 = np.concatenate([faces, faces[:, :1]], axis=1)  # [1280, 4]
    out = []
    for b in range(B):
        vg = vertices[b][faces4]                      # [1280, 4, 3]
        vg4 = np.concatenate(
            [vg, np.ones((NF, 4, 1), dtype=np.float32)], axis=2)  # [1280,4,4]
        out.append(np.ascontiguousarray(
            vg4.transpose(1, 2, 0).reshape(16, NF).astype(np.float32)))
    return out


def kernel(vertices, viewpoints, faces, img_size):
    vertices = np.asarray(vertices, dtype=np.float32)
    viewpoints = np.asarray(viewpoints, dtype=np.float32)
    faces = np.asarray(faces, dtype=np.int32)
    assert int(img_size) == IMG and vertices.shape == (B, V, 3)

    nc = _get_program()
    vgts = _host_layout(vertices, faces)
    xgrid, tbasis, xb65 = _consts()
    in_maps = [
        {"vgt16": vgts[b], "eye": np.ascontiguousarray(viewpoints[b]),
         "xgrid": xgrid, "tbasis": tbasis, "xb65": xb65}
        for b in range(B)
    ]
    res = run_bass_kernel_spmd(nc, in_maps, core_ids=list(range(B)))
    # device pixel order is (j, i): transpose back to raster (i, j)
    sil = np.stack([
        res.results[b]["sil"].reshape(IMG, IMG).T for b in range(B)
    ])
    return sil.reshape(B, 1, IMG, IMG).astype(np.float32)


if __name__ == "__main__":
    rng = np.random.default_rng(0)
    verts = rng.standard_normal((B, V, 3), dtype=np.float32) * 0.5
    vps = rng.standard_normal((B, 3), dtype=np.float32)
    fcs = rng.integers(0, V, (NF, 3), dtype=np.int32)
    out = kernel(verts, vps, fcs, IMG)
    print(out.shape, out.sum())
